# revision 24
# baseline (speedup 1.0000x reference)
"""Trainium2 Bass kernel for nn_DeepBKT (4-layer DeepBKT-style transformer).

Sharding: pure data-parallel over batch. B=32 sequences -> 8 NeuronCores x 4
sequences. Weights replicated. No collectives.

Fast-path design (zero biases / unit LN gains, which setup_inputs produces):
  - Host precompute: x0 = q + pos (f32, residual stream start), yT = (qa+pos)^T
    (bf16), tefr[j, i] = te[i,j] * fr[j] / sqrt(dk) packed causally (bf16).
    All three DMA straight into SBUF and stay resident per sequence; no DRAM
    scratch round-trips.
  - All PE operands bf16 except the f32 residual/LN stream: bf16 is 1 cyc/row
    at every moving size and p-state (fp32r degrades 4x below 256 columns).
  - PE transposes use a bf16 identity as the moving operand (1 cyc/row vs 2.0
    for an f32 identity); the psum->sbuf copy converts to bf16 for the
    downstream matmuls.
  - Per-head softmax works on a causally-packed [128, 1280] row (j-partition,
    i-free): 4 matmuls -> 4 psum->bf16 multiplies by resident tefr -> diagonal
    mask via affine_select -> ONE exp over the packed row.
  - Softmax denominators ride the PV matmul as a ones-column (row 64 of the
    psum); normalization divides ctx rows via partition-broadcast reciprocal.
    Query row 0 (fully masked, 0/0) is repaired with a 4-column memset.
  - W1/W2 are loaded once per layer (bf16, resident), not once per sequence.
  - Software pipeline attn(b) / ffn(b-1) so the FFN matmul stream covers the
    softmax chain latency of the next sequence; PSUM split 3 stream / 3
    scores / 2 ctx banks to avoid cross-phase false serialization.
"""

import sys

for _p in ("/opt/trn_rl_repo",):
    if _p not in sys.path:
        sys.path.insert(0, _p)

from collections import deque

import numpy as np

import concourse.bacc as bacc
import concourse.bass as bass
import concourse.tile as tile
import concourse.mybir as mybir
from concourse.masks import make_identity

import concourse.tile_utils as tile_utils

tile_utils.max_sbuf_usage = 208 * 1024

F32 = mybir.dt.float32
F32R = mybir.dt.float32r
BF16 = mybir.dt.bfloat16
AF = mybir.ActivationFunctionType
ALU = mybir.AluOpType

P = 128
S, D, H, FF = 512, 512, 8, 2048
DK = D // H  # 64
NT = S // P  # 4
DT = D // P  # 4
NKF = FF // P  # 16
EPS = 1e-5
NEG_BIG = -1e30
N_CORES = 8
PB = [0, 512, 896, 1152]  # packed column base per j-tile (cols i >= tj*128)
PACK_W = 1280


def build_fast(L=4, NB=4):
    nc = bacc.Bacc("TRN2", target_bir_lowering=False, debug=False,
                   num_devices=N_CORES)

    x0_d = nc.dram_tensor("x0", [NB, S, D], F32R, kind="ExternalInput")
    yt_d = nc.dram_tensor("yT", [NB, D, S], BF16, kind="ExternalInput")
    tf_d = nc.dram_tensor("tefr", [NB, P, PACK_W], BF16, kind="ExternalInput")
    wk_d = nc.dram_tensor("Wk", [L, D, D], BF16, kind="ExternalInput")
    wv_d = nc.dram_tensor("Wv", [L, D, D], BF16, kind="ExternalInput")
    wo_d = nc.dram_tensor("Wo", [L, D, D], BF16, kind="ExternalInput")
    w1_d = nc.dram_tensor("W1", [L, D, FF], BF16, kind="ExternalInput")
    w2_d = nc.dram_tensor("W2", [L, FF, D], BF16, kind="ExternalInput")
    out_d = nc.dram_tensor("out", [NB, S, D], F32R, kind="ExternalOutput")

    with tile.TileContext(nc) as tc:
        with (
            tc.tile_pool(name="const", bufs=1) as constp,
            tc.tile_pool(name="state", bufs=18) as statep,
            tc.tile_pool(name="ytp", bufs=NB) as ytp,
            tc.tile_pool(name="tfp", bufs=NB) as tfp,
            tc.tile_pool(name="bfp", bufs=5) as bfp,
            tc.tile_pool(name="medp", bufs=3) as medp,
            tc.tile_pool(name="spp", bufs=2) as spp,
            tc.tile_pool(name="etp", bufs=5) as etp,
            tc.tile_pool(name="htp", bufs=18) as htp,
            tc.tile_pool(name="w3p", bufs=6) as w3p,
            tc.tile_pool(name="w1p", bufs=1) as w1p,
            tc.tile_pool(name="w2p", bufs=1) as w2p,
            tc.tile_pool(name="smallp", bufs=8) as smallp,
            tc.tile_pool(name="denp", bufs=2) as denp,
            tc.tile_pool(name="pstr", bufs=3, space="PSUM") as pstr,
            tc.tile_pool(name="psS", bufs=3, space="PSUM") as psS,
            tc.tile_pool(name="psC", bufs=2, space="PSUM") as psC,
        ):
            ident_f = constp.tile([P, P], F32, tag="identf")
            make_identity(nc, ident_f)
            ident_frt = constp.tile([P, P], F32R, tag="identfr")
            nc.scalar.copy(out=ident_frt[:], in_=ident_f[:])
            ident_fr = ident_frt[:]
            eps_t = constp.tile([P, 1], F32, tag="eps")
            nc.vector.memset(eps_t, EPS)
            ones32 = constp.tile([P, NT * H], F32, tag="ones32")
            nc.vector.memset(ones32, 1.0)

            def transpose_to_bf16(src_of_it, use_vector=False):
                """[i, d] f32r tiles -> one [128, DT, 512] bf16 tile [d, i]."""
                dst = bfp.tile([P, DT, S], BF16, tag="bf", name="tpose")
                for c in range(DT):
                    ps = pstr.tile([P, S], F32R, tag="ps", name="tps")
                    for it in range(NT):
                        nc.tensor.transpose(
                            ps[:, it * P:(it + 1) * P],
                            src_of_it(it)[:, c * P:(c + 1) * P],
                            ident_fr,
                        )
                    if use_vector:
                        nc.vector.tensor_copy(out=dst[:, c, :], in_=ps[:])
                    else:
                        nc.scalar.copy(out=dst[:, c, :], in_=ps[:])
                return dst

            def ln_apply(t, rowsum, dst):
                """LayerNorm over free dim (unit gain / zero bias)."""
                mean_neg = smallp.tile([P, 1], F32, tag="mneg")
                nc.scalar.mul(out=mean_neg, in_=rowsum, mul=-1.0 / D)
                var_s = smallp.tile([P, 1], F32, tag="vars")
                nc.scalar.activation(out=dst, in_=t, func=AF.Square,
                                     bias=mean_neg, scale=1.0,
                                     accum_out=var_s)
                std = smallp.tile([P, 1], F32, tag="std")
                nc.scalar.activation(out=std, in_=var_s, func=AF.Sqrt,
                                     bias=eps_t, scale=1.0 / D)
                rstd = smallp.tile([P, 1], F32, tag="rstd")
                nc.vector.reciprocal(out=rstd, in_=std)
                nc.vector.tensor_scalar(out=dst, in0=t, scalar1=mean_neg,
                                        scalar2=rstd, op0=ALU.add,
                                        op1=ALU.mult)

            # ============ per-sequence init ============
            x_tiles = {}
            yts = {}
            tfs = {}
            for b in range(NB):
                xb = []
                for it in range(NT):
                    xt = statep.tile([P, D], F32R, tag="x", name="xt")
                    nc.sync.dma_start(out=xt[:],
                                      in_=x0_d[b, it * P:(it + 1) * P, :])
                    xb.append(xt)
                x_tiles[b] = xb
                yT = ytp.tile([P, DT, S], BF16, tag="yt", name="yT")
                nc.gpsimd.dma_start(
                    out=yT[:], in_=yt_d[b].rearrange("(c p) i -> p c i", p=P))
                yts[b] = yT
                tf = tfp.tile([P, PACK_W], BF16, tag="tf", name="tf")
                nc.gpsimd.dma_start(out=tf[:], in_=tf_d[b])
                tfs[b] = tf

            def emit_scores(b, h, qkT, tf):
                hp0 = (h % 2) * DK
                qh = qkT[hp0:hp0 + DK, h // 2, :]
                sp = spp.tile([P, PACK_W], BF16, tag="sp", name="sp")
                for tj in range(NT):
                    i0 = tj * P
                    ni = S - i0
                    base = PB[tj]
                    sc = psS.tile([P, S], F32, tag="sc", name="sc")
                    nc.tensor.matmul(sc[:, 0:ni], qh[:, i0:i0 + P],
                                     qh[:, i0:S], start=True, stop=True)
                    nc.vector.tensor_mul(out=sp[:, base:base + ni],
                                         in0=sc[:, 0:ni],
                                         in1=tf[:, base:base + ni])
                    # strict causal mask on the diagonal block: keep j < i,
                    # i.e. partition p < free f
                    nc.gpsimd.affine_select(
                        out=sp[:, base:base + P], in_=sp[:, base:base + P],
                        compare_op=ALU.is_gt, fill=NEG_BIG,
                        base=0, channel_multiplier=-1, pattern=[[1, P]])
                eT = etp.tile([P, PACK_W], BF16, tag="et", name="eT")
                nc.scalar.activation(out=eT[:], in_=sp[:], func=AF.Exp)
                return eT

            def emit_pv(b, h, eT, vext, ctxT):
                hp0 = (h % 2) * DK
                ctx = psC.tile([P, S], F32, tag="ctx", name="ctx")
                for tj in range(NT):
                    i0 = tj * P
                    ni = S - i0
                    base = PB[tj]
                    nc.tensor.matmul(ctx[:, i0:S], vext[:, tj, h, :],
                                     eT[:, base:base + ni],
                                     start=(tj == 0), stop=(tj == NT - 1))
                rden = denp.tile([DK, S], F32, tag="den", name="rden")
                nc.vector.reciprocal(out=rden[:], in_=ctx[DK:P, :])
                nc.vector.tensor_mul(out=ctxT[hp0:hp0 + DK, h // 2, :],
                                     in0=ctx[0:DK, :], in1=rden[:])

            vext_init = [0]

            def emit_qkv(b, wk, wv):
                xb = x_tiles[b]
                yT = yts[b]
                xT = transpose_to_bf16(lambda it: xb[it])
                qkT = bfp.tile([P, DT, S], BF16, tag="bf", name="qkT")
                for mt in range(DT):
                    ps = pstr.tile([P, S], F32, tag="ps", name="qps")
                    for c in range(DT):
                        nc.tensor.matmul(ps[:],
                                         wk[:, c, mt * P:(mt + 1) * P],
                                         xT[:, c, :], start=(c == 0),
                                         stop=(c == DT - 1))
                    nc.vector.tensor_copy(out=qkT[:, mt, :], in_=ps[:])
                vext = bfp.tile([P, NT, H, 2 * DK], BF16, tag="vx",
                                name="vext", bufs=2)
                if vext_init[0] < 2:
                    vext_init[0] += 1
                    nc.vector.memset(vext[:, :, :, DK:], 1.0)
                for it in range(NT):
                    ps = pstr.tile([P, S], F32, tag="ps", name="vps")
                    for c in range(DT):
                        nc.tensor.matmul(ps[:],
                                         yT[:, c, it * P:(it + 1) * P],
                                         wv[:, c, :], start=(c == 0),
                                         stop=(c == DT - 1))
                    nc.vector.tensor_copy(
                        out=vext[:, it, :, 0:DK],
                        in_=ps[:].rearrange("p (h k) -> p h k", h=H))
                return qkT, vext

            def emit_ffn1_kf(x1T, w1, kf, hts):
                hps = pstr.tile([P, S], F32, tag="ps", name="hps")
                for c in range(DT):
                    nc.tensor.matmul(hps[:],
                                     w1[:, c, kf * P:(kf + 1) * P],
                                     x1T[:, c, :], start=(c == 0),
                                     stop=(c == DT - 1))
                hT = htp.tile([P, S], BF16, tag="ht", name="hT")
                if kf % 2 == 0:
                    nc.scalar.activation(out=hT[:], in_=hps[:], func=AF.Relu)
                else:
                    nc.vector.tensor_scalar_max(out=hT[:], in0=hps[:],
                                                scalar1=0.0)
                hts.append(hT)

            def emit_y2_block(pb, pl, it, hts, w2t, x1b, x2b):
                y2 = pstr.tile([P, S], F32, tag="ps", name="y2ps")
                for kf in range(NKF):
                    nc.tensor.matmul(y2[:],
                                     hts[kf][:, it * P:(it + 1) * P],
                                     w2t[:, kf, :], start=(kf == 0),
                                     stop=(kf == NKF - 1))
                t2 = medp.tile([P, D], F32, tag="med", name="t2")
                rs2 = smallp.tile([P, 1], F32, tag="rs")
                nc.vector.scalar_tensor_tensor(
                    out=t2[:], in0=y2[:], scalar=1.0, in1=x1b[it][:],
                    op0=ALU.mult, op1=ALU.add, accum_out=rs2)
                x2 = statep.tile([P, D], F32R, tag="x", name="x2")
                ln_apply(t2[:], rs2[:], x2[:])
                x2b.append(x2)
                if pl == L - 1:
                    nc.gpsimd.dma_start(
                        out=out_d[pb, it * P:(it + 1) * P, :], in_=x2[:])

            def emit_part2(b, ctxT, wo):
                xb = x_tiles[b]
                # query row 0 is fully masked: den==0 -> 0/0; reference zeroes
                # the attention row, so ctx row i=0 must be 0.
                nc.vector.memset(ctxT[:, :, 0:1], 0.0)
                x1b = []
                for it in range(NT):
                    ps = pstr.tile([P, S], F32, tag="ps", name="ops")
                    for c in range(DT):
                        nc.tensor.matmul(ps[:],
                                         ctxT[:, c, it * P:(it + 1) * P],
                                         wo[:, c, :], start=(c == 0),
                                         stop=(c == DT - 1))
                    t = medp.tile([P, D], F32, tag="med", name="t1")
                    rs = smallp.tile([P, 1], F32, tag="rs")
                    nc.vector.scalar_tensor_tensor(
                        out=t[:], in0=ps[:], scalar=1.0, in1=xb[it][:],
                        op0=ALU.mult, op1=ALU.add, accum_out=rs)
                    x1 = statep.tile([P, D], F32R, tag="x", name="x1")
                    ln_apply(t[:], rs[:], x1[:])
                    x1b.append(x1)
                x_tiles[b] = x1b

            def load_ffn_w(l):
                w1 = w1p.tile([P, DT, FF], BF16, tag="w1", name="w1")
                nc.sync.dma_start(
                    out=w1[:], in_=w1_d[l].rearrange("(c p) f -> p c f", p=P))
                w2t = w2p.tile([P, NKF, D], BF16, tag="w2", name="w2t")
                nc.sync.dma_start(
                    out=w2t[:], in_=w2_d[l].rearrange("(k p) d -> p k d", p=P))
                return w1, w2t

            # ============ layers: fine-grained attn(b) / ffn(prev) weave ====
            # PE stream per sequence: qkv -> (scores(h) + 2 ffn1 blocks)x8
            # with PV(h-3) woven in -> (y2 block + PV drain)x4 -> out-proj.
            # The previous sequence's FFN matmuls pace the softmax chains so
            # the PE never idles and stays at max p-state.
            prev = None  # (b, l, w1, w2t)
            w1c = w2c = None
            for l in range(L):
                wk = w3p.tile([P, DT, D], BF16, tag="w3", name="wk")
                nc.sync.dma_start(
                    out=wk[:], in_=wk_d[l].rearrange("(c p) m -> p c m", p=P))
                wv = w3p.tile([P, DT, D], BF16, tag="w3", name="wv")
                nc.sync.dma_start(
                    out=wv[:], in_=wv_d[l].rearrange("(c p) m -> p c m", p=P))
                wo = w3p.tile([P, DT, D], BF16, tag="w3", name="wo")
                nc.sync.dma_start(
                    out=wo[:], in_=wo_d[l].rearrange("(c p) m -> p c m", p=P))
                if l == 0:
                    w1c, w2c = load_ffn_w(0)
                for b in range(NB):
                    tf = tfs[b]
                    qkT, vext = emit_qkv(b, wk, wv)
                    if prev is not None:
                        pb, pl, pw1, pw2 = prev
                        px1b = x_tiles[pb]
                        px1T = transpose_to_bf16(lambda it: px1b[it])
                        phts = []
                    ctxT = bfp.tile([P, DT, S], BF16, tag="bf", name="ctxT")
                    eTs = []
                    for h in range(H):
                        eTs.append(emit_scores(b, h, qkT, tf))
                        if prev is not None:
                            emit_ffn1_kf(px1T, pw1, 2 * h, phts)
                            emit_ffn1_kf(px1T, pw1, 2 * h + 1, phts)
                        if h >= 3:
                            emit_pv(b, h - 3, eTs[h - 3], vext, ctxT)
                    px2b = []
                    for it in range(NT):
                        if prev is not None:
                            emit_y2_block(pb, pl, it, phts, pw2, px1b, px2b)
                        if it < 3:
                            emit_pv(b, H - 3 + it, eTs[H - 3 + it], vext,
                                    ctxT)
                    if prev is not None:
                        x_tiles[pb] = px2b
                        if pl == l - 1:
                            # prev was the last seq of layer l-1: its FFN
                            # weights are consumed; load this layer's.
                            w1c, w2c = load_ffn_w(l)
                    emit_part2(b, ctxT, wo)
                    prev = (b, l, w1c, w2c)
            # drain the last sequence's FFN
            pb, pl, pw1, pw2 = prev
            px1b = x_tiles[pb]
            px1T = transpose_to_bf16(lambda it: px1b[it])
            phts = []
            for kf in range(NKF):
                emit_ffn1_kf(px1T, pw1, kf, phts)
            px2b = []
            for it in range(NT):
                emit_y2_block(pb, pl, it, phts, pw2, px1b, px2b)
            x_tiles[pb] = px2b

    nc.compile()
    return nc


# ================== general (non-fast) fallback: baseline kernel ==========

def _baseline_build(L=4, NB=4, fast=True):
    """Build the per-core Bass kernel. fast=True assumes zero biases and
    unit LN gains (checked by the host against the actual input values)."""
    nc = bacc.Bacc("TRN2", target_bir_lowering=False, debug=False,
                   num_devices=N_CORES)

    q_d = nc.dram_tensor("q", [NB, S, D], F32, kind="ExternalInput")
    qa_d = nc.dram_tensor("qa", [NB, S, D], F32, kind="ExternalInput")
    pid_d = nc.dram_tensor("pid", [NB, S, S], F32, kind="ExternalInput")
    fr_d = nc.dram_tensor("fr", [NB, S], F32, kind="ExternalInput")
    pos_d = nc.dram_tensor("pos", [S, D], F32, kind="ExternalInput")
    wk_d = nc.dram_tensor("Wk", [L, D, D], F32R, kind="ExternalInput")
    wv_d = nc.dram_tensor("Wv", [L, D, D], F32R, kind="ExternalInput")
    wo_d = nc.dram_tensor("Wo", [L, D, D], F32R, kind="ExternalInput")
    w1_d = nc.dram_tensor("W1", [L, D, FF], FFN_DT, kind="ExternalInput")
    w2_d = nc.dram_tensor("W2", [L, FF, D], FFN_DT, kind="ExternalInput")
    if not fast:
        bk_d = nc.dram_tensor("bk", [L, D], F32, kind="ExternalInput")
        bv_d = nc.dram_tensor("bv", [L, D], F32, kind="ExternalInput")
        bo_d = nc.dram_tensor("bo", [L, D], F32, kind="ExternalInput")
        b1_d = nc.dram_tensor("b1", [L, FF], F32, kind="ExternalInput")
        b2_d = nc.dram_tensor("b2", [L, D], F32, kind="ExternalInput")
        g1_d = nc.dram_tensor("g1", [L, D], F32, kind="ExternalInput")
        gb1_d = nc.dram_tensor("gb1", [L, D], F32, kind="ExternalInput")
        g2_d = nc.dram_tensor("g2", [L, D], F32, kind="ExternalInput")
        gb2_d = nc.dram_tensor("gb2", [L, D], F32, kind="ExternalInput")
    out_d = nc.dram_tensor("out", [NB, S, D], F32, kind="ExternalOutput")

    with tile.TileContext(nc) as tc:
        with (
            tc.tile_pool(name="const", bufs=1) as constp,
            tc.tile_pool(name="state", bufs=18 if fast else 12) as statep,
            tc.tile_pool(name="big", bufs=6 if fast else 4) as bigp,
            tc.tile_pool(name="med", bufs=22 if fast else 12) as medp,
            tc.tile_pool(name="w3", bufs=4) as w3p,
            tc.tile_pool(name="w1", bufs=2) as w1p,
            tc.tile_pool(name="w2", bufs=2) as w2p,
            tc.tile_pool(name="small", bufs=8) as smallp,
            tc.tile_pool(name="frsp", bufs=NB) as frsp,
            tc.tile_pool(name="ps", bufs=8, space="PSUM") as psp,
            tc.tile_pool(name="dram", bufs=1, space="DRAM") as dramp,
        ):
            ident = constp.tile([P, P], F32, tag="ident")
            make_identity(nc, ident)
            eps_t = constp.tile([P, 1], F32, tag="eps")
            nc.vector.memset(eps_t, EPS)
            eps37 = constp.tile([P, 1], F32, tag="eps37")
            nc.vector.memset(eps37, 1e-37)
            ones32 = constp.tile([P, NT * H], F32, tag="ones32")
            nc.vector.memset(ones32, 1.0)

            teT_dram = dramp.tile([NB, S, S], F32, tag="teT_d")
            yT_dram = dramp.tile([NB, S, D], F32R, tag="yT_d")

            def big_tile(dt_, cols=D):
                return bigp.tile([P, NT, cols], dt_, tag="big", name="bigt")

            def med_tile(dt_, cols=D):
                return medp.tile([P, cols], dt_, tag="med", name="medt")

            def transpose_512(src_of_it, out_dt):
                """src_of_it(it) -> AP [128, 512] seq-major tiles.
                Returns one [128, NT, 512] tile = transposed (feature-major)."""
                dst = big_tile(out_dt)
                for c in range(DT):
                    ps = psp.tile([P, S], F32, tag="psC")
                    for it in range(NT):
                        nc.tensor.transpose(
                            ps[:, it * P:(it + 1) * P],
                            src_of_it(it)[:, c * P:(c + 1) * P],
                            ident,
                        )
                    nc.scalar.copy(out=dst[:, c, :], in_=ps[:])
                return dst

            def ln_apply(t, rowsum, dst, g_bc=None, b_bc=None):
                """LayerNorm over free dim: t [128,512] f32 (pre-LN values),
                rowsum [128,1] = sum over free. Writes normalized into dst."""
                mean_neg = smallp.tile([P, 1], F32, tag="mneg")
                nc.scalar.mul(out=mean_neg, in_=rowsum, mul=-1.0 / D)
                var_s = smallp.tile([P, 1], F32, tag="vars")
                # dst used as throwaway scratch for the squares
                nc.scalar.activation(out=dst, in_=t, func=AF.Square,
                                     bias=mean_neg, scale=1.0,
                                     accum_out=var_s)
                std = smallp.tile([P, 1], F32, tag="std")
                nc.scalar.activation(out=std, in_=var_s, func=AF.Sqrt,
                                     bias=eps_t, scale=1.0 / D)
                rstd = smallp.tile([P, 1], F32, tag="rstd")
                nc.vector.reciprocal(out=rstd, in_=std)
                nc.vector.tensor_scalar(out=dst, in0=t, scalar1=mean_neg,
                                        scalar2=rstd, op0=ALU.add,
                                        op1=ALU.mult)
                if g_bc is not None:
                    nc.vector.tensor_mul(out=dst, in0=dst, in1=g_bc)
                if b_bc is not None:
                    nc.vector.tensor_add(out=dst, in0=dst, in1=b_bc)

            def bcast_row(src_row, cols=D):
                """Broadcast [1, cols] sbuf row to [128, cols] via PE."""
                onecol = constp.tile([1, P], F32, tag="onecol")
                nc.vector.memset(onecol, 1.0)
                ps = psp.tile([P, cols], F32, tag="psC")
                nc.tensor.matmul(ps[:], onecol[:], src_row, start=True,
                                 stop=True)
                dst = medp.tile([P, cols], F32, tag="bc", bufs=10, name="bct")
                nc.vector.tensor_copy(out=dst[:], in_=ps[:])
                return dst

            # ============ per-sequence init ============
            x_tiles = {}   # b -> list of NT state APs [128, 512] f32 (seq-major)
            frs = {}
            pos_t = big_tile(F32)
            nc.sync.dma_start(
                out=pos_t[:],
                in_=pos_d[:].rearrange("(it p) d -> p it d", p=P))

            for b in range(NB):
                # x = q + pos
                qt = big_tile(F32)
                nc.sync.dma_start(
                    out=qt[:], in_=q_d[b].rearrange("(it p) d -> p it d", p=P))
                xb = []
                for it in range(NT):
                    xt = statep.tile([P, D], F32, tag="x")
                    nc.vector.tensor_add(out=xt[:], in0=qt[:, it, :],
                                         in1=pos_t[:, it, :])
                    xb.append(xt)
                x_tiles[b] = xb

                # y = qa + pos; yT -> DRAM scratch (f32r)
                yt = big_tile(F32)
                nc.sync.dma_start(
                    out=yt[:], in_=qa_d[b].rearrange("(it p) d -> p it d", p=P))
                for it in range(NT):
                    nc.vector.tensor_add(out=yt[:, it, :], in0=yt[:, it, :],
                                         in1=pos_t[:, it, :])
                yT = transpose_512(lambda it: yt[:, it, :], F32R)
                nc.sync.dma_start(
                    out=yT_dram[b].rearrange("(c p) i -> p c i", p=P),
                    in_=yT[:])

                # te = exp(sigmoid(pid)); teT -> DRAM scratch (f32)
                pt = big_tile(F32, S)
                nc.sync.dma_start(
                    out=pt[:],
                    in_=pid_d[b].rearrange("(it p) j -> p it j", p=P))
                for it in range(NT):
                    nc.scalar.activation(out=pt[:, it, :], in_=pt[:, it, :],
                                         func=AF.Sigmoid)
                    nc.scalar.activation(out=pt[:, it, :], in_=pt[:, it, :],
                                         func=AF.Exp)
                teT = transpose_512(lambda it: pt[:, it, :], F32)
                nc.sync.dma_start(
                    out=teT_dram[b].rearrange("(c p) i -> p c i", p=P),
                    in_=teT[:])

                # forget gate, pre-scaled by 1/sqrt(DK)
                ft = frsp.tile([P, NT], F32, tag="frs")
                nc.sync.dma_start(
                    out=ft[:], in_=fr_d[b].rearrange("(t p) -> p t", p=P))
                nc.scalar.mul(out=ft[:], in_=ft[:], mul=1.0 / np.sqrt(DK))
                frs[b] = ft

            # ============ layers ============
            for l in range(L):
                wk = w3p.tile([P, DT, D], F32R, tag="w3")
                nc.sync.dma_start(
                    out=wk[:], in_=wk_d[l].rearrange("(c p) m -> p c m", p=P))
                wv = w3p.tile([P, DT, D], F32R, tag="w3")
                nc.sync.dma_start(
                    out=wv[:], in_=wv_d[l].rearrange("(c p) m -> p c m", p=P))
                wo = w3p.tile([P, DT, D], F32R, tag="w3")
                nc.sync.dma_start(
                    out=wo[:], in_=wo_d[l].rearrange("(c p) m -> p c m", p=P))

                if not fast:
                    bk_sb = smallp.tile([P, DT], F32, tag="bk")
                    nc.sync.dma_start(
                        out=bk_sb[:],
                        in_=bk_d[l].rearrange("(c p) -> p c", p=P))
                    row = smallp.tile([1, D], F32, tag="brow", bufs=2)
                    nc.sync.dma_start(out=row[:], in_=bv_d[l][None, :])
                    bv_bc = bcast_row(row[:])
                    row2 = smallp.tile([1, D], F32, tag="brow", bufs=2)
                    nc.sync.dma_start(out=row2[:], in_=bo_d[l][None, :])
                    bo_bc = bcast_row(row2[:])
                    row3 = smallp.tile([1, D], F32, tag="brow", bufs=2)
                    nc.sync.dma_start(out=row3[:], in_=b2_d[l][None, :])
                    b2_bc = bcast_row(row3[:])
                    b1_sb = smallp.tile([P, NKF], F32, tag="b1")
                    nc.sync.dma_start(
                        out=b1_sb[:],
                        in_=b1_d[l].rearrange("(c p) -> p c", p=P))
                    rg1 = smallp.tile([1, D], F32, tag="brow", bufs=2)
                    nc.sync.dma_start(out=rg1[:], in_=g1_d[l][None, :])
                    g1_bc = bcast_row(rg1[:])
                    rb1 = smallp.tile([1, D], F32, tag="brow", bufs=2)
                    nc.sync.dma_start(out=rb1[:], in_=gb1_d[l][None, :])
                    gb1_bc = bcast_row(rb1[:])
                    rg2 = smallp.tile([1, D], F32, tag="brow", bufs=2)
                    nc.sync.dma_start(out=rg2[:], in_=g2_d[l][None, :])
                    g2_bc = bcast_row(rg2[:])
                    rb2 = smallp.tile([1, D], F32, tag="brow", bufs=2)
                    nc.sync.dma_start(out=rb2[:], in_=gb2_d[l][None, :])
                    gb2_bc = bcast_row(rb2[:])
                else:
                    bk_sb = None
                    bv_bc = bo_bc = b2_bc = None
                    b1_sb = None
                    g1_bc = gb1_bc = g2_bc = gb2_bc = None

                # -------- attention phase --------
                def emit_scores(b, qkT, teT, h):
                    hp0 = (h % 2) * DK
                    qh = qkT[hp0:hp0 + DK, h // 2, :]
                    eTs = []
                    for tj in range(NT):
                        i0 = tj * P
                        ni = S - i0
                        sc_ps = psp.tile([P, S], F32, tag="psC", name="scps")
                        nc.tensor.matmul(
                            sc_ps[:, 0:ni], qh[:, i0:i0 + P], qh[:, i0:S],
                            start=True, stop=True)
                        sp = med_tile(F32)
                        nc.vector.scalar_tensor_tensor(
                            out=sp[:, 0:ni], in0=sc_ps[:, 0:ni],
                            scalar=frs[b][:, tj:tj + 1],
                            in1=teT[:, tj, i0:S],
                            op0=ALU.mult, op1=ALU.mult)
                        # strict causal mask on the diagonal block:
                        # keep j < i, i.e. partition p < free f
                        nc.gpsimd.affine_select(
                            out=sp[:, 0:P], in_=sp[:, 0:P],
                            compare_op=ALU.is_gt, fill=NEG_BIG,
                            base=0, channel_multiplier=-1,
                            pattern=[[1, P]])
                        eT = med_tile(F32R)
                        nc.scalar.activation(out=eT[:, 0:ni],
                                             in_=sp[:, 0:ni], func=AF.Exp)
                        eTs.append(eT)
                    return eTs

                def emit_pv(vext, ctxT, h, eTs):
                    hp0 = (h % 2) * DK
                    ctx_ps = psp.tile([P, S], F32, tag="psC", name="ctxps")
                    for tj in range(NT):
                        i0 = tj * P
                        ni = S - i0
                        nc.tensor.matmul(
                            ctx_ps[0:DK + 1, i0:S],
                            vext[:, tj, h, :], eTs[tj][:, 0:ni],
                            start=(tj == 0), stop=(tj == NT - 1))
                    dtmp = smallp.tile([1, S], F32, tag="dtmp", bufs=2)
                    nc.scalar.activation(
                        out=dtmp[:], in_=ctx_ps[DK:DK + 1, :],
                        func=AF.Identity, bias=eps37[0:1])
                    denB = smallp.tile([DK, S], F32, tag="dinvB", bufs=3)
                    nc.gpsimd.partition_broadcast(denB[:], dtmp[:])
                    dinvB = smallp.tile([DK, S], F32, tag="dinvB", bufs=3)
                    nc.vector.reciprocal_approx_fast(out=dinvB[:],
                                                     in_=denB[:])
                    nc.vector.tensor_mul(
                        out=ctxT[hp0:hp0 + DK, h // 2, :],
                        in0=ctx_ps[0:DK, :], in1=dinvB[:])

                for b in range(NB):
                    xb = x_tiles[b]
                    # prefetch the per-seq DRAM-scratch streams first so the
                    # DMAs overlap the transpose + projection matmuls
                    teT = big_tile(F32, S)
                    nc.gpsimd.dma_start(
                        out=teT[:],
                        in_=teT_dram[b].rearrange("(c p) i -> p c i", p=P))
                    yT = big_tile(F32R)
                    nc.gpsimd.dma_start(
                        out=yT[:],
                        in_=yT_dram[b].rearrange("(c p) i -> p c i", p=P))
                    xT = transpose_512(lambda it: xb[it], F32R)

                    # qkT[d, i] feature-major
                    qkT = big_tile(F32R)
                    for mt in range(DT):
                        ps = psp.tile([P, S], F32, tag="psC")
                        for c in range(DT):
                            nc.tensor.matmul(
                                ps[:], wk[:, c, mt * P:(mt + 1) * P],
                                xT[:, c, :], start=(c == 0),
                                stop=(c == DT - 1))
                        if bk_sb is not None:
                            nc.scalar.activation(
                                out=qkT[:, mt, :], in_=ps[:],
                                func=AF.Identity, bias=bk_sb[:, mt:mt + 1])
                        else:
                            nc.vector.tensor_copy(out=qkT[:, mt, :], in_=ps[:])

                    # v seq-major with ones column per head: [128, it, h, 65]
                    vext = bigp.tile([P, NT, H, DK + 1], F32R, tag="big")
                    nc.scalar.copy(
                        out=vext[:, :, :, DK:DK + 1],
                        in_=ones32[:].rearrange("p (a b o) -> p a b o",
                                                a=NT, b=H, o=1))
                    for it in range(NT):
                        ps = psp.tile([P, S], F32, tag="psC")
                        for c in range(DT):
                            nc.tensor.matmul(
                                ps[:], yT[:, c, it * P:(it + 1) * P],
                                wv[:, c, :], start=(c == 0),
                                stop=(c == DT - 1))
                        pv = ps[:].rearrange("p (h k) -> p h k", h=H)
                        if bv_bc is not None:
                            nc.vector.scalar_tensor_tensor(
                                out=vext[:, it, :, 0:DK], in0=pv, scalar=1.0,
                                in1=bv_bc[:].rearrange("p (h k) -> p h k",
                                                       h=H),
                                op0=ALU.mult, op1=ALU.add)
                        else:
                            nc.vector.tensor_copy(out=vext[:, it, :, 0:DK],
                                                  in_=pv)

                    ctxT = big_tile(F32R)
                    from collections import deque
                    pending = deque()
                    for h in range(H):
                        pending.append((h, emit_scores(b, qkT, teT, h)))
                        if len(pending) > 3:
                            ph, peTs = pending.popleft()
                            emit_pv(vext, ctxT, ph, peTs)
                    while pending:
                        ph, peTs = pending.popleft()
                        emit_pv(vext, ctxT, ph, peTs)

                    # out-proj + residual + LN1
                    x1b = []
                    for it in range(NT):
                        ps = psp.tile([P, S], F32, tag="psC")
                        for c in range(DT):
                            nc.tensor.matmul(
                                ps[:], ctxT[:, c, it * P:(it + 1) * P],
                                wo[:, c, :], start=(c == 0),
                                stop=(c == DT - 1))
                        t = med_tile(F32)
                        rs = smallp.tile([P, 1], F32, tag="rs")
                        if bo_bc is not None:
                            nc.vector.scalar_tensor_tensor(
                                out=t[:], in0=ps[:], scalar=1.0, in1=bo_bc[:],
                                op0=ALU.mult, op1=ALU.add)
                            nc.vector.scalar_tensor_tensor(
                                out=t[:], in0=t[:], scalar=1.0, in1=xb[it][:],
                                op0=ALU.mult, op1=ALU.add, accum_out=rs)
                        else:
                            nc.vector.scalar_tensor_tensor(
                                out=t[:], in0=ps[:], scalar=1.0,
                                in1=xb[it][:],
                                op0=ALU.mult, op1=ALU.add, accum_out=rs)
                        x1 = statep.tile([P, D], F32, tag="x")
                        ln_apply(t[:], rs[:], x1[:], g1_bc and g1_bc[:],
                                 gb1_bc and gb1_bc[:])
                        x1b.append(x1)
                    x_tiles[b] = x1b

                # -------- FFN phase --------
                for b in range(NB):
                    x1b = x_tiles[b]
                    x1T = transpose_512(lambda it: x1b[it], FFN_DT)
                    y2_ps = [psp.tile([P, S], F32, tag="psC", name="y2ps")
                             for _i in range(NT)]
                    pend_ffn2 = []
                    for g in range(NKF // 4):
                        w1g = w1p.tile([P, DT, 4 * P], FFN_DT, tag="w1")
                        nc.sync.dma_start(
                            out=w1g[:],
                            in_=w1_d[l].rearrange("(c p) f -> p c f",
                                                  p=P)[:, :,
                                                       g * 512:(g + 1) * 512])
                        w2g = w2p.tile([P, 4, D], FFN_DT, tag="w2")
                        nc.sync.dma_start(
                            out=w2g[:],
                            in_=w2_d[l].rearrange("(c p) d -> p c d",
                                                  p=P)[:, 4 * g:4 * g + 4, :])
                        for j in range(4):
                            kf = 4 * g + j
                            h_ps = psp.tile([P, S], F32, tag="psC")
                            for c in range(DT):
                                nc.tensor.matmul(
                                    h_ps[:], w1g[:, c, j * P:(j + 1) * P],
                                    x1T[:, c, :], start=(c == 0),
                                    stop=(c == DT - 1))
                            hT = med_tile(FFN_DT)
                            if b1_sb is not None:
                                nc.scalar.activation(
                                    out=hT[:], in_=h_ps[:], func=AF.Relu,
                                    bias=b1_sb[:, kf:kf + 1])
                            elif kf % 2 == 0:
                                nc.scalar.activation(out=hT[:], in_=h_ps[:],
                                                     func=AF.Relu)
                            else:
                                nc.vector.tensor_scalar_max(
                                    out=hT[:], in0=h_ps[:], scalar1=0.0)
                            # pipeline by two kf: ffn2(kf-2) is emitted
                            # after ffn1(kf) so the PE isn't stalled on relu
                            pend_ffn2.append((hT, w2g, j, kf))
                            if len(pend_ffn2) > 2:
                                phT, pw2g, pj, pkf = pend_ffn2.pop(0)
                                for it in range(NT):
                                    nc.tensor.matmul(
                                        y2_ps[it][:],
                                        phT[:, it * P:(it + 1) * P],
                                        pw2g[:, pj, :], start=(pkf == 0),
                                        stop=(pkf == NKF - 1))
                    for phT, pw2g, pj, pkf in pend_ffn2:
                        for it in range(NT):
                            nc.tensor.matmul(
                                y2_ps[it][:], phT[:, it * P:(it + 1) * P],
                                pw2g[:, pj, :], start=(pkf == 0),
                                stop=(pkf == NKF - 1))
                    x2b = []
                    for it in range(NT):
                        t2 = med_tile(F32)
                        rs2 = smallp.tile([P, 1], F32, tag="rs")
                        if b2_bc is not None:
                            nc.vector.scalar_tensor_tensor(
                                out=t2[:], in0=y2_ps[it][:], scalar=1.0,
                                in1=b2_bc[:], op0=ALU.mult, op1=ALU.add)
                            nc.vector.scalar_tensor_tensor(
                                out=t2[:], in0=t2[:], scalar=1.0,
                                in1=x1b[it][:], op0=ALU.mult, op1=ALU.add,
                                accum_out=rs2)
                        else:
                            nc.vector.scalar_tensor_tensor(
                                out=t2[:], in0=y2_ps[it][:], scalar=1.0,
                                in1=x1b[it][:], op0=ALU.mult, op1=ALU.add,
                                accum_out=rs2)
                        x2 = statep.tile([P, D], F32, tag="x")
                        ln_apply(t2[:], rs2[:], x2[:], g2_bc and g2_bc[:],
                                 gb2_bc and gb2_bc[:])
                        x2b.append(x2)
                    x_tiles[b] = x2b
                    if l == L - 1:
                        for it in range(NT):
                            nc.sync.dma_start(
                                out=out_d[b, it * P:(it + 1) * P, :],
                                in_=x2b[it][:])

    nc.compile()
    return nc


def build_general(L=4, NB=4):
    return _baseline_build(L, NB, fast=False)


_BUILD_CACHE = {}


def _get_nc(L, NB, fast):
    key = (L, NB, fast)
    if key not in _BUILD_CACHE:
        _BUILD_CACHE[key] = (build_fast(L, NB) if fast
                             else build_general(L, NB))
    return _BUILD_CACHE[key]


def _is_fast(w):
    return (all(np.all(np.asarray(w[n]) == 0.0) for n in
                ["bk", "bv", "bo", "b1", "b2", "ln1_b", "ln2_b"])
            and all(np.all(np.asarray(w[n]) == 1.0)
                    for n in ["ln1_g", "ln2_g"]))


def make_in_maps(inputs, L=4, NB=4, n_cores=N_CORES):
    """Shard full inputs into per-core in_maps. Returns (in_maps, fast)."""
    import ml_dtypes
    bf = ml_dtypes.bfloat16
    f32 = np.float32
    fast = _is_fast(inputs)
    if not fast:
        return _baseline_make_in_maps(inputs, L=L, NB=NB, n_cores=n_cores)

    q = np.asarray(inputs["q_embed_data"], f32)
    qa = np.asarray(inputs["qa_embed_data"], f32)
    pid = np.asarray(inputs["pid_embed_data"], f32)
    fr = np.asarray(inputs["forget_rate"], f32)[:, :, 0]
    pos = np.asarray(inputs["pos_emb"], f32).reshape(S, D)
    B = q.shape[0]

    x0 = np.ascontiguousarray(q + pos[None])
    yT = np.ascontiguousarray(
        np.swapaxes(qa + pos[None], 1, 2)).astype(bf)
    te = np.clip(np.exp(1.0 / (1.0 + np.exp(-pid))), 1e-5, 1e5)
    # tefr[b, j, i] = te[b, i, j] * fr[b, j] / sqrt(DK), causally packed
    tf_ji = np.swapaxes(te, 1, 2) * (fr[:, :, None] * (1.0 / np.sqrt(DK)))
    tfP = np.zeros((B, P, PACK_W), f32)
    for tj in range(NT):
        i0 = tj * P
        tfP[:, :, PB[tj]:PB[tj] + S - i0] = \
            tf_ji[:, i0:i0 + P, i0:].astype(f32)
    tfP = np.ascontiguousarray(tfP).astype(bf)

    wts = {n: np.ascontiguousarray(np.asarray(inputs[n], f32)[:L]).astype(bf)
           for n in ["Wk", "Wv", "Wo", "W1", "W2"]}

    in_maps = []
    for c in range(n_cores):
        sl = slice(c * NB, (c + 1) * NB)
        m = {
            "x0": x0[sl], "yT": yT[sl], "tefr": tfP[sl],
            "Wk": wts["Wk"], "Wv": wts["Wv"], "Wo": wts["Wo"],
            "W1": wts["W1"], "W2": wts["W2"],
        }
        in_maps.append(m)
    return in_maps, fast


def kernel(**inputs):
    from concourse.bass_utils import run_bass_kernel_spmd

    B = int(np.asarray(inputs["q_embed_data"]).shape[0])
    NB = B // N_CORES
    L = int(np.asarray(inputs["Wk"]).shape[0])
    in_maps, fast = make_in_maps(inputs, L=L, NB=NB)
    nc = _get_nc(L, NB, fast)
    res = run_bass_kernel_spmd(nc, in_maps, core_ids=list(range(N_CORES)))
    out = np.concatenate([res.results[c]["out"] for c in range(N_CORES)],
                         axis=0)
    return out.astype(np.float32)


def _baseline_make_in_maps(inputs, L=4, NB=4, n_cores=N_CORES):
    """Shard full inputs into per-core in_maps. Returns (in_maps, fast)."""
    f32 = np.float32
    q = np.ascontiguousarray(np.asarray(inputs["q_embed_data"], f32))
    qa = np.ascontiguousarray(np.asarray(inputs["qa_embed_data"], f32))
    pid = np.ascontiguousarray(np.asarray(inputs["pid_embed_data"], f32))
    fr = np.ascontiguousarray(np.asarray(inputs["forget_rate"], f32)[:, :, 0])
    pos = np.ascontiguousarray(np.asarray(inputs["pos_emb"], f32)[0])
    names = ["Wk", "bk", "Wv", "bv", "Wo", "bo", "ln1_g", "ln1_b", "W1", "b1",
             "W2", "b2", "ln2_g", "ln2_b"]
    w = {n: np.ascontiguousarray(np.asarray(inputs[n], f32)) for n in names}
    if FFN_BF16:
        import ml_dtypes
        w["W1"] = w["W1"].astype(ml_dtypes.bfloat16)
        w["W2"] = w["W2"].astype(ml_dtypes.bfloat16)

    fast = (all(np.all(w[n] == 0.0) for n in
                ["bk", "bv", "bo", "b1", "b2", "ln1_b", "ln2_b"])
            and all(np.all(w[n] == 1.0) for n in ["ln1_g", "ln2_g"]))

    in_maps = []
    for c in range(n_cores):
        sl = slice(c * NB, (c + 1) * NB)
        m = {
            "q": q[sl], "qa": qa[sl], "pid": pid[sl], "fr": fr[sl],
            "pos": pos,
            "Wk": w["Wk"][:L], "Wv": w["Wv"][:L], "Wo": w["Wo"][:L],
            "W1": w["W1"][:L], "W2": w["W2"][:L],
        }
        if not fast:
            m.update({
                "bk": w["bk"][:L], "bv": w["bv"][:L], "bo": w["bo"][:L],
                "b1": w["b1"][:L], "b2": w["b2"][:L],
                "g1": w["ln1_g"][:L], "gb1": w["ln1_b"][:L],
                "g2": w["ln2_g"][:L], "gb2": w["ln2_b"][:L],
            })
        in_maps.append(m)
    return in_maps, fast


# revision 25
# speedup vs baseline: 1.2782x; 1.2782x over previous
"""Trainium2 Bass kernel for nn_DeepBKT (4-layer DeepBKT-style transformer).

Sharding: pure data-parallel over batch. B=32 sequences -> 8 NeuronCores x 4
sequences. Weights replicated. No collectives.

Fast-path design (zero biases / unit LN gains, which setup_inputs produces):
  - Host precompute: x0 = q + pos (f32, residual stream start), yT = (qa+pos)^T
    (bf16), tefr[j, i] = te[i,j] * fr[j] / sqrt(dk) packed causally (bf16).
    All three DMA straight into SBUF and stay resident per sequence; no DRAM
    scratch round-trips.
  - All PE operands bf16 except the f32 residual/LN stream: bf16 is 1 cyc/row
    at every moving size and p-state (fp32r degrades 4x below 256 columns).
  - PE transposes use a bf16 identity as the moving operand (1 cyc/row vs 2.0
    for an f32 identity); the psum->sbuf copy converts to bf16 for the
    downstream matmuls.
  - Per-head softmax works on a causally-packed [128, 1280] row (j-partition,
    i-free): 4 matmuls -> 4 psum->bf16 multiplies by resident tefr -> diagonal
    mask via affine_select -> ONE exp over the packed row.
  - Softmax denominators ride the PV matmul as a ones-column (row 64 of the
    psum); normalization divides ctx rows via partition-broadcast reciprocal.
    Query row 0 (fully masked, 0/0) is repaired with a 4-column memset.
  - W1/W2 are loaded once per layer (bf16, resident), not once per sequence.
  - Software pipeline attn(b) / ffn(b-1) so the FFN matmul stream covers the
    softmax chain latency of the next sequence; PSUM split 3 stream / 3
    scores / 2 ctx banks to avoid cross-phase false serialization.
"""

import sys

for _p in ("/opt/trn_rl_repo",):
    if _p not in sys.path:
        sys.path.insert(0, _p)

from collections import deque

import numpy as np

import concourse.bacc as bacc
import concourse.bass as bass
import concourse.tile as tile
import concourse.mybir as mybir
from concourse.masks import make_identity

import concourse.tile_utils as tile_utils

tile_utils.max_sbuf_usage = 208 * 1024

F32 = mybir.dt.float32
F32R = mybir.dt.float32r
BF16 = mybir.dt.bfloat16
AF = mybir.ActivationFunctionType
ALU = mybir.AluOpType

P = 128
S, D, H, FF = 512, 512, 8, 2048
DK = D // H  # 64
NT = S // P  # 4
DT = D // P  # 4
NKF = FF // P  # 16
EPS = 1e-5
NEG_BIG = -1e30
N_CORES = 8
PB = [0, 512, 896, 1152]  # packed column base per j-tile (cols i >= tj*128)
PACK_W = 1280


def build_fast(L=4, NB=4):
    nc = bacc.Bacc("TRN2", target_bir_lowering=False, debug=False,
                   num_devices=N_CORES)

    x0_d = nc.dram_tensor("x0", [NB, S, D], F32R, kind="ExternalInput")
    yt_d = nc.dram_tensor("yT", [NB, D, S], BF16, kind="ExternalInput")
    tf_d = nc.dram_tensor("tefr", [NB, P, PACK_W], BF16, kind="ExternalInput")
    wk_d = nc.dram_tensor("Wk", [L, D, D], BF16, kind="ExternalInput")
    wv_d = nc.dram_tensor("Wv", [L, D, D], BF16, kind="ExternalInput")
    wo_d = nc.dram_tensor("Wo", [L, D, D], BF16, kind="ExternalInput")
    w1_d = nc.dram_tensor("W1", [L, D, FF], BF16, kind="ExternalInput")
    w2_d = nc.dram_tensor("W2", [L, FF, D], BF16, kind="ExternalInput")
    out_d = nc.dram_tensor("out", [NB, S, D], F32R, kind="ExternalOutput")

    with tile.TileContext(nc) as tc:
        with (
            tc.tile_pool(name="const", bufs=1) as constp,
            tc.tile_pool(name="state", bufs=18) as statep,
            tc.tile_pool(name="ytp", bufs=NB) as ytp,
            tc.tile_pool(name="tfp", bufs=NB) as tfp,
            tc.tile_pool(name="bfp", bufs=5) as bfp,
            tc.tile_pool(name="medp", bufs=3) as medp,
            tc.tile_pool(name="spp", bufs=2) as spp,
            tc.tile_pool(name="etp", bufs=5) as etp,
            tc.tile_pool(name="htp", bufs=18) as htp,
            tc.tile_pool(name="w3p", bufs=6) as w3p,
            tc.tile_pool(name="w1p", bufs=1) as w1p,
            tc.tile_pool(name="w2p", bufs=1) as w2p,
            tc.tile_pool(name="smallp", bufs=8) as smallp,
            tc.tile_pool(name="denp", bufs=2) as denp,
            tc.tile_pool(name="pstr", bufs=3, space="PSUM") as pstr,
            tc.tile_pool(name="psS", bufs=3, space="PSUM") as psS,
            tc.tile_pool(name="psC", bufs=2, space="PSUM") as psC,
        ):
            ident_f = constp.tile([P, P], F32, tag="identf")
            make_identity(nc, ident_f)
            ident_frt = constp.tile([P, P], F32R, tag="identfr")
            nc.scalar.copy(out=ident_frt[:], in_=ident_f[:])
            ident_fr = ident_frt[:]
            eps_t = constp.tile([P, 1], F32, tag="eps")
            nc.vector.memset(eps_t, EPS)
            ones32 = constp.tile([P, NT * H], F32, tag="ones32")
            nc.vector.memset(ones32, 1.0)

            def transpose_to_bf16(src_of_it, use_vector=False):
                """[i, d] f32r tiles -> one [128, DT, 512] bf16 tile [d, i]."""
                dst = bfp.tile([P, DT, S], BF16, tag="bf", name="tpose")
                for c in range(DT):
                    ps = pstr.tile([P, S], F32R, tag="ps", name="tps")
                    for it in range(NT):
                        nc.tensor.transpose(
                            ps[:, it * P:(it + 1) * P],
                            src_of_it(it)[:, c * P:(c + 1) * P],
                            ident_fr,
                        )
                    if use_vector:
                        nc.vector.tensor_copy(out=dst[:, c, :], in_=ps[:])
                    else:
                        nc.scalar.copy(out=dst[:, c, :], in_=ps[:])
                return dst

            def ln_apply(t, rowsum, dst):
                """LayerNorm over free dim (unit gain / zero bias)."""
                mean_neg = smallp.tile([P, 1], F32, tag="mneg")
                nc.scalar.mul(out=mean_neg, in_=rowsum, mul=-1.0 / D)
                var_s = smallp.tile([P, 1], F32, tag="vars")
                nc.scalar.activation(out=dst, in_=t, func=AF.Square,
                                     bias=mean_neg, scale=1.0,
                                     accum_out=var_s)
                std = smallp.tile([P, 1], F32, tag="std")
                nc.scalar.activation(out=std, in_=var_s, func=AF.Sqrt,
                                     bias=eps_t, scale=1.0 / D)
                rstd = smallp.tile([P, 1], F32, tag="rstd")
                nc.vector.reciprocal(out=rstd, in_=std)
                nc.vector.tensor_scalar(out=dst, in0=t, scalar1=mean_neg,
                                        scalar2=rstd, op0=ALU.add,
                                        op1=ALU.mult)

            # ============ per-sequence init ============
            x_tiles = {}
            yts = {}
            tfs = {}
            for b in range(NB):
                xb = []
                for it in range(NT):
                    xt = statep.tile([P, D], F32R, tag="x", name="xt")
                    nc.sync.dma_start(out=xt[:],
                                      in_=x0_d[b, it * P:(it + 1) * P, :])
                    xb.append(xt)
                x_tiles[b] = xb
                yT = ytp.tile([P, DT, S], BF16, tag="yt", name="yT")
                nc.gpsimd.dma_start(
                    out=yT[:], in_=yt_d[b].rearrange("(c p) i -> p c i", p=P))
                yts[b] = yT
                tf = tfp.tile([P, PACK_W], BF16, tag="tf", name="tf")
                nc.gpsimd.dma_start(out=tf[:], in_=tf_d[b])
                tfs[b] = tf

            def emit_scores(b, h, qkT, tf):
                hp0 = (h % 2) * DK
                qh = qkT[hp0:hp0 + DK, h // 2, :]
                sp = spp.tile([P, PACK_W], BF16, tag="sp", name="sp")
                for tj in range(NT):
                    i0 = tj * P
                    ni = S - i0
                    base = PB[tj]
                    sc = psS.tile([P, S], F32, tag="sc", name="sc")
                    nc.tensor.matmul(sc[:, 0:ni], qh[:, i0:i0 + P],
                                     qh[:, i0:S], start=True, stop=True)
                    nc.vector.tensor_mul(out=sp[:, base:base + ni],
                                         in0=sc[:, 0:ni],
                                         in1=tf[:, base:base + ni])
                    # strict causal mask on the diagonal block: keep j < i,
                    # i.e. partition p < free f
                    nc.gpsimd.affine_select(
                        out=sp[:, base:base + P], in_=sp[:, base:base + P],
                        compare_op=ALU.is_gt, fill=NEG_BIG,
                        base=0, channel_multiplier=-1, pattern=[[1, P]])
                eT = etp.tile([P, PACK_W], BF16, tag="et", name="eT")
                nc.scalar.activation(out=eT[:], in_=sp[:], func=AF.Exp)
                return eT

            def emit_pv(b, h, eT, vext, ctxT):
                hp0 = (h % 2) * DK
                ctx = psC.tile([P, S], F32, tag="ctx", name="ctx")
                for tj in range(NT):
                    i0 = tj * P
                    ni = S - i0
                    base = PB[tj]
                    nc.tensor.matmul(ctx[:, i0:S], vext[:, tj, h, :],
                                     eT[:, base:base + ni],
                                     start=(tj == 0), stop=(tj == NT - 1))
                denr = denp.tile([DK, S], F32, tag="drw", name="denr",
                                 bufs=2)
                nc.scalar.copy(out=denr[:], in_=ctx[DK:P, :])
                rden = denp.tile([DK, S], F32, tag="den", name="rden")
                nc.vector.reciprocal_approx_fast(out=rden[:], in_=denr[:])
                nc.vector.tensor_mul(out=ctxT[hp0:hp0 + DK, h // 2, :],
                                     in0=ctx[0:DK, :], in1=rden[:])

            vext_init = [0]

            def emit_qkv(b, wk, wv):
                xb = x_tiles[b]
                yT = yts[b]
                xT = transpose_to_bf16(lambda it: xb[it])
                qkT = bfp.tile([P, DT, S], BF16, tag="bf", name="qkT")
                for mt in range(DT):
                    ps = pstr.tile([P, S], F32, tag="ps", name="qps")
                    for c in range(DT):
                        nc.tensor.matmul(ps[:],
                                         wk[:, c, mt * P:(mt + 1) * P],
                                         xT[:, c, :], start=(c == 0),
                                         stop=(c == DT - 1))
                    nc.scalar.copy(out=qkT[:, mt, :], in_=ps[:])
                vext = bfp.tile([P, NT, H, 2 * DK], BF16, tag="vx",
                                name="vext", bufs=2)
                if vext_init[0] < 2:
                    vext_init[0] += 1
                    nc.vector.memset(vext[:, :, :, DK:], 1.0)
                for it in range(NT):
                    ps = pstr.tile([P, S], F32, tag="ps", name="vps")
                    for c in range(DT):
                        nc.tensor.matmul(ps[:],
                                         yT[:, c, it * P:(it + 1) * P],
                                         wv[:, c, :], start=(c == 0),
                                         stop=(c == DT - 1))
                    nc.vector.tensor_copy(
                        out=vext[:, it, :, 0:DK],
                        in_=ps[:].rearrange("p (h k) -> p h k", h=H))
                return qkT, vext

            def emit_ffn1_kf(x1T, w1, kf, hts):
                hps = pstr.tile([P, S], F32, tag="ps", name="hps")
                for c in range(DT):
                    nc.tensor.matmul(hps[:],
                                     w1[:, c, kf * P:(kf + 1) * P],
                                     x1T[:, c, :], start=(c == 0),
                                     stop=(c == DT - 1))
                hT = htp.tile([P, S], BF16, tag="ht", name="hT")
                if kf % 2 == 0:
                    nc.scalar.activation(out=hT[:], in_=hps[:], func=AF.Relu)
                else:
                    nc.vector.tensor_scalar_max(out=hT[:], in0=hps[:],
                                                scalar1=0.0)
                hts.append(hT)

            def emit_y2_block(pb, pl, it, hts, w2t, x1b, x2b):
                y2 = pstr.tile([P, S], F32, tag="ps", name="y2ps")
                for kf in range(NKF):
                    nc.tensor.matmul(y2[:],
                                     hts[kf][:, it * P:(it + 1) * P],
                                     w2t[:, kf, :], start=(kf == 0),
                                     stop=(kf == NKF - 1))
                t2 = medp.tile([P, D], F32, tag="med", name="t2")
                rs2 = smallp.tile([P, 1], F32, tag="rs")
                nc.vector.scalar_tensor_tensor(
                    out=t2[:], in0=y2[:], scalar=1.0, in1=x1b[it][:],
                    op0=ALU.mult, op1=ALU.add, accum_out=rs2)
                x2 = statep.tile([P, D], F32R, tag="x", name="x2")
                ln_apply(t2[:], rs2[:], x2[:])
                x2b.append(x2)
                if pl == L - 1:
                    nc.gpsimd.dma_start(
                        out=out_d[pb, it * P:(it + 1) * P, :], in_=x2[:])

            def emit_part2(b, ctxT, wo):
                xb = x_tiles[b]
                # query row 0 is fully masked: den==0 -> 0/0; reference zeroes
                # the attention row, so ctx row i=0 must be 0.
                nc.vector.memset(ctxT[:, :, 0:1], 0.0)
                x1b = []
                for it in range(NT):
                    ps = pstr.tile([P, S], F32, tag="ps", name="ops")
                    for c in range(DT):
                        nc.tensor.matmul(ps[:],
                                         ctxT[:, c, it * P:(it + 1) * P],
                                         wo[:, c, :], start=(c == 0),
                                         stop=(c == DT - 1))
                    t = medp.tile([P, D], F32, tag="med", name="t1")
                    rs = smallp.tile([P, 1], F32, tag="rs")
                    nc.vector.scalar_tensor_tensor(
                        out=t[:], in0=ps[:], scalar=1.0, in1=xb[it][:],
                        op0=ALU.mult, op1=ALU.add, accum_out=rs)
                    x1 = statep.tile([P, D], F32R, tag="x", name="x1")
                    ln_apply(t[:], rs[:], x1[:])
                    x1b.append(x1)
                x_tiles[b] = x1b

            def load_ffn_w(l):
                w1 = w1p.tile([P, DT, FF], BF16, tag="w1", name="w1")
                nc.sync.dma_start(
                    out=w1[:], in_=w1_d[l].rearrange("(c p) f -> p c f", p=P))
                w2t = w2p.tile([P, NKF, D], BF16, tag="w2", name="w2t")
                nc.sync.dma_start(
                    out=w2t[:], in_=w2_d[l].rearrange("(k p) d -> p k d", p=P))
                return w1, w2t

            # ============ layers: fine-grained attn(b) / ffn(prev) weave ====
            # PE stream per sequence: qkv -> (scores(h) + 2 ffn1 blocks)x8
            # with PV(h-3) woven in -> (y2 block + PV drain)x4 -> out-proj.
            # The previous sequence's FFN matmuls pace the softmax chains so
            # the PE never idles and stays at max p-state.
            prev = None  # (b, l, w1, w2t)
            w1c = w2c = None
            for l in range(L):
                wk = w3p.tile([P, DT, D], BF16, tag="w3", name="wk")
                nc.sync.dma_start(
                    out=wk[:], in_=wk_d[l].rearrange("(c p) m -> p c m", p=P))
                wv = w3p.tile([P, DT, D], BF16, tag="w3", name="wv")
                nc.sync.dma_start(
                    out=wv[:], in_=wv_d[l].rearrange("(c p) m -> p c m", p=P))
                wo = w3p.tile([P, DT, D], BF16, tag="w3", name="wo")
                nc.sync.dma_start(
                    out=wo[:], in_=wo_d[l].rearrange("(c p) m -> p c m", p=P))
                if l == 0:
                    w1c, w2c = load_ffn_w(0)
                for b in range(NB):
                    tf = tfs[b]
                    qkT, vext = emit_qkv(b, wk, wv)
                    if prev is not None:
                        pb, pl, pw1, pw2 = prev
                        px1b = x_tiles[pb]
                        px1T = transpose_to_bf16(lambda it: px1b[it])
                        phts = []
                    ctxT = bfp.tile([P, DT, S], BF16, tag="bf", name="ctxT")
                    eTs = []
                    for h in range(H):
                        eTs.append(emit_scores(b, h, qkT, tf))
                        if prev is not None:
                            emit_ffn1_kf(px1T, pw1, 2 * h, phts)
                            emit_ffn1_kf(px1T, pw1, 2 * h + 1, phts)
                        if h >= 3:
                            emit_pv(b, h - 3, eTs[h - 3], vext, ctxT)
                    px2b = []
                    for it in range(NT):
                        if prev is not None:
                            emit_y2_block(pb, pl, it, phts, pw2, px1b, px2b)
                        if it < 3:
                            emit_pv(b, H - 3 + it, eTs[H - 3 + it], vext,
                                    ctxT)
                    if prev is not None:
                        x_tiles[pb] = px2b
                        if pl == l - 1:
                            # prev was the last seq of layer l-1: its FFN
                            # weights are consumed; load this layer's.
                            w1c, w2c = load_ffn_w(l)
                    emit_part2(b, ctxT, wo)
                    prev = (b, l, w1c, w2c)
            # drain the last sequence's FFN
            pb, pl, pw1, pw2 = prev
            px1b = x_tiles[pb]
            px1T = transpose_to_bf16(lambda it: px1b[it])
            phts = []
            for kf in range(NKF):
                emit_ffn1_kf(px1T, pw1, kf, phts)
            px2b = []
            for it in range(NT):
                emit_y2_block(pb, pl, it, phts, pw2, px1b, px2b)
            x_tiles[pb] = px2b

    nc.compile()
    return nc


# ================== general (non-fast) fallback: baseline kernel ==========

def _baseline_build(L=4, NB=4, fast=True):
    """Build the per-core Bass kernel. fast=True assumes zero biases and
    unit LN gains (checked by the host against the actual input values)."""
    nc = bacc.Bacc("TRN2", target_bir_lowering=False, debug=False,
                   num_devices=N_CORES)

    q_d = nc.dram_tensor("q", [NB, S, D], F32, kind="ExternalInput")
    qa_d = nc.dram_tensor("qa", [NB, S, D], F32, kind="ExternalInput")
    pid_d = nc.dram_tensor("pid", [NB, S, S], F32, kind="ExternalInput")
    fr_d = nc.dram_tensor("fr", [NB, S], F32, kind="ExternalInput")
    pos_d = nc.dram_tensor("pos", [S, D], F32, kind="ExternalInput")
    wk_d = nc.dram_tensor("Wk", [L, D, D], F32R, kind="ExternalInput")
    wv_d = nc.dram_tensor("Wv", [L, D, D], F32R, kind="ExternalInput")
    wo_d = nc.dram_tensor("Wo", [L, D, D], F32R, kind="ExternalInput")
    w1_d = nc.dram_tensor("W1", [L, D, FF], FFN_DT, kind="ExternalInput")
    w2_d = nc.dram_tensor("W2", [L, FF, D], FFN_DT, kind="ExternalInput")
    if not fast:
        bk_d = nc.dram_tensor("bk", [L, D], F32, kind="ExternalInput")
        bv_d = nc.dram_tensor("bv", [L, D], F32, kind="ExternalInput")
        bo_d = nc.dram_tensor("bo", [L, D], F32, kind="ExternalInput")
        b1_d = nc.dram_tensor("b1", [L, FF], F32, kind="ExternalInput")
        b2_d = nc.dram_tensor("b2", [L, D], F32, kind="ExternalInput")
        g1_d = nc.dram_tensor("g1", [L, D], F32, kind="ExternalInput")
        gb1_d = nc.dram_tensor("gb1", [L, D], F32, kind="ExternalInput")
        g2_d = nc.dram_tensor("g2", [L, D], F32, kind="ExternalInput")
        gb2_d = nc.dram_tensor("gb2", [L, D], F32, kind="ExternalInput")
    out_d = nc.dram_tensor("out", [NB, S, D], F32, kind="ExternalOutput")

    with tile.TileContext(nc) as tc:
        with (
            tc.tile_pool(name="const", bufs=1) as constp,
            tc.tile_pool(name="state", bufs=18 if fast else 12) as statep,
            tc.tile_pool(name="big", bufs=6 if fast else 4) as bigp,
            tc.tile_pool(name="med", bufs=22 if fast else 12) as medp,
            tc.tile_pool(name="w3", bufs=4) as w3p,
            tc.tile_pool(name="w1", bufs=2) as w1p,
            tc.tile_pool(name="w2", bufs=2) as w2p,
            tc.tile_pool(name="small", bufs=8) as smallp,
            tc.tile_pool(name="frsp", bufs=NB) as frsp,
            tc.tile_pool(name="ps", bufs=8, space="PSUM") as psp,
            tc.tile_pool(name="dram", bufs=1, space="DRAM") as dramp,
        ):
            ident = constp.tile([P, P], F32, tag="ident")
            make_identity(nc, ident)
            eps_t = constp.tile([P, 1], F32, tag="eps")
            nc.vector.memset(eps_t, EPS)
            eps37 = constp.tile([P, 1], F32, tag="eps37")
            nc.vector.memset(eps37, 1e-37)
            ones32 = constp.tile([P, NT * H], F32, tag="ones32")
            nc.vector.memset(ones32, 1.0)

            teT_dram = dramp.tile([NB, S, S], F32, tag="teT_d")
            yT_dram = dramp.tile([NB, S, D], F32R, tag="yT_d")

            def big_tile(dt_, cols=D):
                return bigp.tile([P, NT, cols], dt_, tag="big", name="bigt")

            def med_tile(dt_, cols=D):
                return medp.tile([P, cols], dt_, tag="med", name="medt")

            def transpose_512(src_of_it, out_dt):
                """src_of_it(it) -> AP [128, 512] seq-major tiles.
                Returns one [128, NT, 512] tile = transposed (feature-major)."""
                dst = big_tile(out_dt)
                for c in range(DT):
                    ps = psp.tile([P, S], F32, tag="psC")
                    for it in range(NT):
                        nc.tensor.transpose(
                            ps[:, it * P:(it + 1) * P],
                            src_of_it(it)[:, c * P:(c + 1) * P],
                            ident,
                        )
                    nc.scalar.copy(out=dst[:, c, :], in_=ps[:])
                return dst

            def ln_apply(t, rowsum, dst, g_bc=None, b_bc=None):
                """LayerNorm over free dim: t [128,512] f32 (pre-LN values),
                rowsum [128,1] = sum over free. Writes normalized into dst."""
                mean_neg = smallp.tile([P, 1], F32, tag="mneg")
                nc.scalar.mul(out=mean_neg, in_=rowsum, mul=-1.0 / D)
                var_s = smallp.tile([P, 1], F32, tag="vars")
                # dst used as throwaway scratch for the squares
                nc.scalar.activation(out=dst, in_=t, func=AF.Square,
                                     bias=mean_neg, scale=1.0,
                                     accum_out=var_s)
                std = smallp.tile([P, 1], F32, tag="std")
                nc.scalar.activation(out=std, in_=var_s, func=AF.Sqrt,
                                     bias=eps_t, scale=1.0 / D)
                rstd = smallp.tile([P, 1], F32, tag="rstd")
                nc.vector.reciprocal(out=rstd, in_=std)
                nc.vector.tensor_scalar(out=dst, in0=t, scalar1=mean_neg,
                                        scalar2=rstd, op0=ALU.add,
                                        op1=ALU.mult)
                if g_bc is not None:
                    nc.vector.tensor_mul(out=dst, in0=dst, in1=g_bc)
                if b_bc is not None:
                    nc.vector.tensor_add(out=dst, in0=dst, in1=b_bc)

            def bcast_row(src_row, cols=D):
                """Broadcast [1, cols] sbuf row to [128, cols] via PE."""
                onecol = constp.tile([1, P], F32, tag="onecol")
                nc.vector.memset(onecol, 1.0)
                ps = psp.tile([P, cols], F32, tag="psC")
                nc.tensor.matmul(ps[:], onecol[:], src_row, start=True,
                                 stop=True)
                dst = medp.tile([P, cols], F32, tag="bc", bufs=10, name="bct")
                nc.vector.tensor_copy(out=dst[:], in_=ps[:])
                return dst

            # ============ per-sequence init ============
            x_tiles = {}   # b -> list of NT state APs [128, 512] f32 (seq-major)
            frs = {}
            pos_t = big_tile(F32)
            nc.sync.dma_start(
                out=pos_t[:],
                in_=pos_d[:].rearrange("(it p) d -> p it d", p=P))

            for b in range(NB):
                # x = q + pos
                qt = big_tile(F32)
                nc.sync.dma_start(
                    out=qt[:], in_=q_d[b].rearrange("(it p) d -> p it d", p=P))
                xb = []
                for it in range(NT):
                    xt = statep.tile([P, D], F32, tag="x")
                    nc.vector.tensor_add(out=xt[:], in0=qt[:, it, :],
                                         in1=pos_t[:, it, :])
                    xb.append(xt)
                x_tiles[b] = xb

                # y = qa + pos; yT -> DRAM scratch (f32r)
                yt = big_tile(F32)
                nc.sync.dma_start(
                    out=yt[:], in_=qa_d[b].rearrange("(it p) d -> p it d", p=P))
                for it in range(NT):
                    nc.vector.tensor_add(out=yt[:, it, :], in0=yt[:, it, :],
                                         in1=pos_t[:, it, :])
                yT = transpose_512(lambda it: yt[:, it, :], F32R)
                nc.sync.dma_start(
                    out=yT_dram[b].rearrange("(c p) i -> p c i", p=P),
                    in_=yT[:])

                # te = exp(sigmoid(pid)); teT -> DRAM scratch (f32)
                pt = big_tile(F32, S)
                nc.sync.dma_start(
                    out=pt[:],
                    in_=pid_d[b].rearrange("(it p) j -> p it j", p=P))
                for it in range(NT):
                    nc.scalar.activation(out=pt[:, it, :], in_=pt[:, it, :],
                                         func=AF.Sigmoid)
                    nc.scalar.activation(out=pt[:, it, :], in_=pt[:, it, :],
                                         func=AF.Exp)
                teT = transpose_512(lambda it: pt[:, it, :], F32)
                nc.sync.dma_start(
                    out=teT_dram[b].rearrange("(c p) i -> p c i", p=P),
                    in_=teT[:])

                # forget gate, pre-scaled by 1/sqrt(DK)
                ft = frsp.tile([P, NT], F32, tag="frs")
                nc.sync.dma_start(
                    out=ft[:], in_=fr_d[b].rearrange("(t p) -> p t", p=P))
                nc.scalar.mul(out=ft[:], in_=ft[:], mul=1.0 / np.sqrt(DK))
                frs[b] = ft

            # ============ layers ============
            for l in range(L):
                wk = w3p.tile([P, DT, D], F32R, tag="w3")
                nc.sync.dma_start(
                    out=wk[:], in_=wk_d[l].rearrange("(c p) m -> p c m", p=P))
                wv = w3p.tile([P, DT, D], F32R, tag="w3")
                nc.sync.dma_start(
                    out=wv[:], in_=wv_d[l].rearrange("(c p) m -> p c m", p=P))
                wo = w3p.tile([P, DT, D], F32R, tag="w3")
                nc.sync.dma_start(
                    out=wo[:], in_=wo_d[l].rearrange("(c p) m -> p c m", p=P))

                if not fast:
                    bk_sb = smallp.tile([P, DT], F32, tag="bk")
                    nc.sync.dma_start(
                        out=bk_sb[:],
                        in_=bk_d[l].rearrange("(c p) -> p c", p=P))
                    row = smallp.tile([1, D], F32, tag="brow", bufs=2)
                    nc.sync.dma_start(out=row[:], in_=bv_d[l][None, :])
                    bv_bc = bcast_row(row[:])
                    row2 = smallp.tile([1, D], F32, tag="brow", bufs=2)
                    nc.sync.dma_start(out=row2[:], in_=bo_d[l][None, :])
                    bo_bc = bcast_row(row2[:])
                    row3 = smallp.tile([1, D], F32, tag="brow", bufs=2)
                    nc.sync.dma_start(out=row3[:], in_=b2_d[l][None, :])
                    b2_bc = bcast_row(row3[:])
                    b1_sb = smallp.tile([P, NKF], F32, tag="b1")
                    nc.sync.dma_start(
                        out=b1_sb[:],
                        in_=b1_d[l].rearrange("(c p) -> p c", p=P))
                    rg1 = smallp.tile([1, D], F32, tag="brow", bufs=2)
                    nc.sync.dma_start(out=rg1[:], in_=g1_d[l][None, :])
                    g1_bc = bcast_row(rg1[:])
                    rb1 = smallp.tile([1, D], F32, tag="brow", bufs=2)
                    nc.sync.dma_start(out=rb1[:], in_=gb1_d[l][None, :])
                    gb1_bc = bcast_row(rb1[:])
                    rg2 = smallp.tile([1, D], F32, tag="brow", bufs=2)
                    nc.sync.dma_start(out=rg2[:], in_=g2_d[l][None, :])
                    g2_bc = bcast_row(rg2[:])
                    rb2 = smallp.tile([1, D], F32, tag="brow", bufs=2)
                    nc.sync.dma_start(out=rb2[:], in_=gb2_d[l][None, :])
                    gb2_bc = bcast_row(rb2[:])
                else:
                    bk_sb = None
                    bv_bc = bo_bc = b2_bc = None
                    b1_sb = None
                    g1_bc = gb1_bc = g2_bc = gb2_bc = None

                # -------- attention phase --------
                def emit_scores(b, qkT, teT, h):
                    hp0 = (h % 2) * DK
                    qh = qkT[hp0:hp0 + DK, h // 2, :]
                    eTs = []
                    for tj in range(NT):
                        i0 = tj * P
                        ni = S - i0
                        sc_ps = psp.tile([P, S], F32, tag="psC", name="scps")
                        nc.tensor.matmul(
                            sc_ps[:, 0:ni], qh[:, i0:i0 + P], qh[:, i0:S],
                            start=True, stop=True)
                        sp = med_tile(F32)
                        nc.vector.scalar_tensor_tensor(
                            out=sp[:, 0:ni], in0=sc_ps[:, 0:ni],
                            scalar=frs[b][:, tj:tj + 1],
                            in1=teT[:, tj, i0:S],
                            op0=ALU.mult, op1=ALU.mult)
                        # strict causal mask on the diagonal block:
                        # keep j < i, i.e. partition p < free f
                        nc.gpsimd.affine_select(
                            out=sp[:, 0:P], in_=sp[:, 0:P],
                            compare_op=ALU.is_gt, fill=NEG_BIG,
                            base=0, channel_multiplier=-1,
                            pattern=[[1, P]])
                        eT = med_tile(F32R)
                        nc.scalar.activation(out=eT[:, 0:ni],
                                             in_=sp[:, 0:ni], func=AF.Exp)
                        eTs.append(eT)
                    return eTs

                def emit_pv(vext, ctxT, h, eTs):
                    hp0 = (h % 2) * DK
                    ctx_ps = psp.tile([P, S], F32, tag="psC", name="ctxps")
                    for tj in range(NT):
                        i0 = tj * P
                        ni = S - i0
                        nc.tensor.matmul(
                            ctx_ps[0:DK + 1, i0:S],
                            vext[:, tj, h, :], eTs[tj][:, 0:ni],
                            start=(tj == 0), stop=(tj == NT - 1))
                    dtmp = smallp.tile([1, S], F32, tag="dtmp", bufs=2)
                    nc.scalar.activation(
                        out=dtmp[:], in_=ctx_ps[DK:DK + 1, :],
                        func=AF.Identity, bias=eps37[0:1])
                    denB = smallp.tile([DK, S], F32, tag="dinvB", bufs=3)
                    nc.gpsimd.partition_broadcast(denB[:], dtmp[:])
                    dinvB = smallp.tile([DK, S], F32, tag="dinvB", bufs=3)
                    nc.vector.reciprocal_approx_fast(out=dinvB[:],
                                                     in_=denB[:])
                    nc.vector.tensor_mul(
                        out=ctxT[hp0:hp0 + DK, h // 2, :],
                        in0=ctx_ps[0:DK, :], in1=dinvB[:])

                for b in range(NB):
                    xb = x_tiles[b]
                    # prefetch the per-seq DRAM-scratch streams first so the
                    # DMAs overlap the transpose + projection matmuls
                    teT = big_tile(F32, S)
                    nc.gpsimd.dma_start(
                        out=teT[:],
                        in_=teT_dram[b].rearrange("(c p) i -> p c i", p=P))
                    yT = big_tile(F32R)
                    nc.gpsimd.dma_start(
                        out=yT[:],
                        in_=yT_dram[b].rearrange("(c p) i -> p c i", p=P))
                    xT = transpose_512(lambda it: xb[it], F32R)

                    # qkT[d, i] feature-major
                    qkT = big_tile(F32R)
                    for mt in range(DT):
                        ps = psp.tile([P, S], F32, tag="psC")
                        for c in range(DT):
                            nc.tensor.matmul(
                                ps[:], wk[:, c, mt * P:(mt + 1) * P],
                                xT[:, c, :], start=(c == 0),
                                stop=(c == DT - 1))
                        if bk_sb is not None:
                            nc.scalar.activation(
                                out=qkT[:, mt, :], in_=ps[:],
                                func=AF.Identity, bias=bk_sb[:, mt:mt + 1])
                        else:
                            nc.scalar.copy(out=qkT[:, mt, :], in_=ps[:])

                    # v seq-major with ones column per head: [128, it, h, 65]
                    vext = bigp.tile([P, NT, H, DK + 1], F32R, tag="big")
                    nc.scalar.copy(
                        out=vext[:, :, :, DK:DK + 1],
                        in_=ones32[:].rearrange("p (a b o) -> p a b o",
                                                a=NT, b=H, o=1))
                    for it in range(NT):
                        ps = psp.tile([P, S], F32, tag="psC")
                        for c in range(DT):
                            nc.tensor.matmul(
                                ps[:], yT[:, c, it * P:(it + 1) * P],
                                wv[:, c, :], start=(c == 0),
                                stop=(c == DT - 1))
                        pv = ps[:].rearrange("p (h k) -> p h k", h=H)
                        if bv_bc is not None:
                            nc.vector.scalar_tensor_tensor(
                                out=vext[:, it, :, 0:DK], in0=pv, scalar=1.0,
                                in1=bv_bc[:].rearrange("p (h k) -> p h k",
                                                       h=H),
                                op0=ALU.mult, op1=ALU.add)
                        else:
                            nc.vector.tensor_copy(out=vext[:, it, :, 0:DK],
                                                  in_=pv)

                    ctxT = big_tile(F32R)
                    from collections import deque
                    pending = deque()
                    for h in range(H):
                        pending.append((h, emit_scores(b, qkT, teT, h)))
                        if len(pending) > 3:
                            ph, peTs = pending.popleft()
                            emit_pv(vext, ctxT, ph, peTs)
                    while pending:
                        ph, peTs = pending.popleft()
                        emit_pv(vext, ctxT, ph, peTs)

                    # out-proj + residual + LN1
                    x1b = []
                    for it in range(NT):
                        ps = psp.tile([P, S], F32, tag="psC")
                        for c in range(DT):
                            nc.tensor.matmul(
                                ps[:], ctxT[:, c, it * P:(it + 1) * P],
                                wo[:, c, :], start=(c == 0),
                                stop=(c == DT - 1))
                        t = med_tile(F32)
                        rs = smallp.tile([P, 1], F32, tag="rs")
                        if bo_bc is not None:
                            nc.vector.scalar_tensor_tensor(
                                out=t[:], in0=ps[:], scalar=1.0, in1=bo_bc[:],
                                op0=ALU.mult, op1=ALU.add)
                            nc.vector.scalar_tensor_tensor(
                                out=t[:], in0=t[:], scalar=1.0, in1=xb[it][:],
                                op0=ALU.mult, op1=ALU.add, accum_out=rs)
                        else:
                            nc.vector.scalar_tensor_tensor(
                                out=t[:], in0=ps[:], scalar=1.0,
                                in1=xb[it][:],
                                op0=ALU.mult, op1=ALU.add, accum_out=rs)
                        x1 = statep.tile([P, D], F32, tag="x")
                        ln_apply(t[:], rs[:], x1[:], g1_bc and g1_bc[:],
                                 gb1_bc and gb1_bc[:])
                        x1b.append(x1)
                    x_tiles[b] = x1b

                # -------- FFN phase --------
                for b in range(NB):
                    x1b = x_tiles[b]
                    x1T = transpose_512(lambda it: x1b[it], FFN_DT)
                    y2_ps = [psp.tile([P, S], F32, tag="psC", name="y2ps")
                             for _i in range(NT)]
                    pend_ffn2 = []
                    for g in range(NKF // 4):
                        w1g = w1p.tile([P, DT, 4 * P], FFN_DT, tag="w1")
                        nc.sync.dma_start(
                            out=w1g[:],
                            in_=w1_d[l].rearrange("(c p) f -> p c f",
                                                  p=P)[:, :,
                                                       g * 512:(g + 1) * 512])
                        w2g = w2p.tile([P, 4, D], FFN_DT, tag="w2")
                        nc.sync.dma_start(
                            out=w2g[:],
                            in_=w2_d[l].rearrange("(c p) d -> p c d",
                                                  p=P)[:, 4 * g:4 * g + 4, :])
                        for j in range(4):
                            kf = 4 * g + j
                            h_ps = psp.tile([P, S], F32, tag="psC")
                            for c in range(DT):
                                nc.tensor.matmul(
                                    h_ps[:], w1g[:, c, j * P:(j + 1) * P],
                                    x1T[:, c, :], start=(c == 0),
                                    stop=(c == DT - 1))
                            hT = med_tile(FFN_DT)
                            if b1_sb is not None:
                                nc.scalar.activation(
                                    out=hT[:], in_=h_ps[:], func=AF.Relu,
                                    bias=b1_sb[:, kf:kf + 1])
                            elif kf % 2 == 0:
                                nc.scalar.activation(out=hT[:], in_=h_ps[:],
                                                     func=AF.Relu)
                            else:
                                nc.vector.tensor_scalar_max(
                                    out=hT[:], in0=h_ps[:], scalar1=0.0)
                            # pipeline by two kf: ffn2(kf-2) is emitted
                            # after ffn1(kf) so the PE isn't stalled on relu
                            pend_ffn2.append((hT, w2g, j, kf))
                            if len(pend_ffn2) > 2:
                                phT, pw2g, pj, pkf = pend_ffn2.pop(0)
                                for it in range(NT):
                                    nc.tensor.matmul(
                                        y2_ps[it][:],
                                        phT[:, it * P:(it + 1) * P],
                                        pw2g[:, pj, :], start=(pkf == 0),
                                        stop=(pkf == NKF - 1))
                    for phT, pw2g, pj, pkf in pend_ffn2:
                        for it in range(NT):
                            nc.tensor.matmul(
                                y2_ps[it][:], phT[:, it * P:(it + 1) * P],
                                pw2g[:, pj, :], start=(pkf == 0),
                                stop=(pkf == NKF - 1))
                    x2b = []
                    for it in range(NT):
                        t2 = med_tile(F32)
                        rs2 = smallp.tile([P, 1], F32, tag="rs")
                        if b2_bc is not None:
                            nc.vector.scalar_tensor_tensor(
                                out=t2[:], in0=y2_ps[it][:], scalar=1.0,
                                in1=b2_bc[:], op0=ALU.mult, op1=ALU.add)
                            nc.vector.scalar_tensor_tensor(
                                out=t2[:], in0=t2[:], scalar=1.0,
                                in1=x1b[it][:], op0=ALU.mult, op1=ALU.add,
                                accum_out=rs2)
                        else:
                            nc.vector.scalar_tensor_tensor(
                                out=t2[:], in0=y2_ps[it][:], scalar=1.0,
                                in1=x1b[it][:], op0=ALU.mult, op1=ALU.add,
                                accum_out=rs2)
                        x2 = statep.tile([P, D], F32, tag="x")
                        ln_apply(t2[:], rs2[:], x2[:], g2_bc and g2_bc[:],
                                 gb2_bc and gb2_bc[:])
                        x2b.append(x2)
                    x_tiles[b] = x2b
                    if l == L - 1:
                        for it in range(NT):
                            nc.sync.dma_start(
                                out=out_d[b, it * P:(it + 1) * P, :],
                                in_=x2b[it][:])

    nc.compile()
    return nc


def build_general(L=4, NB=4):
    return _baseline_build(L, NB, fast=False)


_BUILD_CACHE = {}


def _get_nc(L, NB, fast):
    key = (L, NB, fast)
    if key not in _BUILD_CACHE:
        _BUILD_CACHE[key] = (build_fast(L, NB) if fast
                             else build_general(L, NB))
    return _BUILD_CACHE[key]


def _is_fast(w):
    return (all(np.all(np.asarray(w[n]) == 0.0) for n in
                ["bk", "bv", "bo", "b1", "b2", "ln1_b", "ln2_b"])
            and all(np.all(np.asarray(w[n]) == 1.0)
                    for n in ["ln1_g", "ln2_g"]))


def make_in_maps(inputs, L=4, NB=4, n_cores=N_CORES):
    """Shard full inputs into per-core in_maps. Returns (in_maps, fast)."""
    import ml_dtypes
    bf = ml_dtypes.bfloat16
    f32 = np.float32
    fast = _is_fast(inputs)
    if not fast:
        return _baseline_make_in_maps(inputs, L=L, NB=NB, n_cores=n_cores)

    q = np.asarray(inputs["q_embed_data"], f32)
    qa = np.asarray(inputs["qa_embed_data"], f32)
    pid = np.asarray(inputs["pid_embed_data"], f32)
    fr = np.asarray(inputs["forget_rate"], f32)[:, :, 0]
    pos = np.asarray(inputs["pos_emb"], f32).reshape(S, D)
    B = q.shape[0]

    x0 = np.ascontiguousarray(q + pos[None])
    yT = np.ascontiguousarray(
        np.swapaxes(qa + pos[None], 1, 2)).astype(bf)
    te = np.clip(np.exp(1.0 / (1.0 + np.exp(-pid))), 1e-5, 1e5)
    # tefr[b, j, i] = te[b, i, j] * fr[b, j] / sqrt(DK), causally packed
    tf_ji = np.swapaxes(te, 1, 2) * (fr[:, :, None] * (1.0 / np.sqrt(DK)))
    tfP = np.zeros((B, P, PACK_W), f32)
    for tj in range(NT):
        i0 = tj * P
        tfP[:, :, PB[tj]:PB[tj] + S - i0] = \
            tf_ji[:, i0:i0 + P, i0:].astype(f32)
    tfP = np.ascontiguousarray(tfP).astype(bf)

    wts = {n: np.ascontiguousarray(np.asarray(inputs[n], f32)[:L]).astype(bf)
           for n in ["Wk", "Wv", "Wo", "W1", "W2"]}

    in_maps = []
    for c in range(n_cores):
        sl = slice(c * NB, (c + 1) * NB)
        m = {
            "x0": x0[sl], "yT": yT[sl], "tefr": tfP[sl],
            "Wk": wts["Wk"], "Wv": wts["Wv"], "Wo": wts["Wo"],
            "W1": wts["W1"], "W2": wts["W2"],
        }
        in_maps.append(m)
    return in_maps, fast


def kernel(**inputs):
    from concourse.bass_utils import run_bass_kernel_spmd

    B = int(np.asarray(inputs["q_embed_data"]).shape[0])
    NB = B // N_CORES
    L = int(np.asarray(inputs["Wk"]).shape[0])
    in_maps, fast = make_in_maps(inputs, L=L, NB=NB)
    nc = _get_nc(L, NB, fast)
    res = run_bass_kernel_spmd(nc, in_maps, core_ids=list(range(N_CORES)))
    out = np.concatenate([res.results[c]["out"] for c in range(N_CORES)],
                         axis=0)
    return out.astype(np.float32)


def _baseline_make_in_maps(inputs, L=4, NB=4, n_cores=N_CORES):
    """Shard full inputs into per-core in_maps. Returns (in_maps, fast)."""
    f32 = np.float32
    q = np.ascontiguousarray(np.asarray(inputs["q_embed_data"], f32))
    qa = np.ascontiguousarray(np.asarray(inputs["qa_embed_data"], f32))
    pid = np.ascontiguousarray(np.asarray(inputs["pid_embed_data"], f32))
    fr = np.ascontiguousarray(np.asarray(inputs["forget_rate"], f32)[:, :, 0])
    pos = np.ascontiguousarray(np.asarray(inputs["pos_emb"], f32)[0])
    names = ["Wk", "bk", "Wv", "bv", "Wo", "bo", "ln1_g", "ln1_b", "W1", "b1",
             "W2", "b2", "ln2_g", "ln2_b"]
    w = {n: np.ascontiguousarray(np.asarray(inputs[n], f32)) for n in names}
    if FFN_BF16:
        import ml_dtypes
        w["W1"] = w["W1"].astype(ml_dtypes.bfloat16)
        w["W2"] = w["W2"].astype(ml_dtypes.bfloat16)

    fast = (all(np.all(w[n] == 0.0) for n in
                ["bk", "bv", "bo", "b1", "b2", "ln1_b", "ln2_b"])
            and all(np.all(w[n] == 1.0) for n in ["ln1_g", "ln2_g"]))

    in_maps = []
    for c in range(n_cores):
        sl = slice(c * NB, (c + 1) * NB)
        m = {
            "q": q[sl], "qa": qa[sl], "pid": pid[sl], "fr": fr[sl],
            "pos": pos,
            "Wk": w["Wk"][:L], "Wv": w["Wv"][:L], "Wo": w["Wo"][:L],
            "W1": w["W1"][:L], "W2": w["W2"][:L],
        }
        if not fast:
            m.update({
                "bk": w["bk"][:L], "bv": w["bv"][:L], "bo": w["bo"][:L],
                "b1": w["b1"][:L], "b2": w["b2"][:L],
                "g1": w["ln1_g"][:L], "gb1": w["ln1_b"][:L],
                "g2": w["ln2_g"][:L], "gb2": w["ln2_b"][:L],
            })
        in_maps.append(m)
    return in_maps, fast


# revision 26
# speedup vs baseline: 1.3212x; 1.0336x over previous
"""Trainium2 Bass kernel for nn_DeepBKT (4-layer DeepBKT-style transformer).

Sharding: pure data-parallel over batch. B=32 sequences -> 8 NeuronCores x 4
sequences. Weights replicated. No collectives.

Fast-path design (zero biases / unit LN gains, which setup_inputs produces):
  - Host precompute: x0 = q + pos (f32, residual stream start), yT = (qa+pos)^T
    (bf16), tefr[j, i] = te[i,j] * fr[j] / sqrt(dk) packed causally (bf16).
    All three DMA straight into SBUF and stay resident per sequence; no DRAM
    scratch round-trips.
  - All PE operands bf16 except the f32 residual/LN stream: bf16 is 1 cyc/row
    at every moving size and p-state (fp32r degrades 4x below 256 columns).
  - PE transposes use a bf16 identity as the moving operand (1 cyc/row vs 2.0
    for an f32 identity); the psum->sbuf copy converts to bf16 for the
    downstream matmuls.
  - Per-head softmax works on a causally-packed [128, 1280] row (j-partition,
    i-free): 4 matmuls -> 4 psum->bf16 multiplies by resident tefr -> diagonal
    mask via affine_select -> ONE exp over the packed row.
  - Softmax denominators ride the PV matmul as a ones-column (row 64 of the
    psum); normalization divides ctx rows via partition-broadcast reciprocal.
    Query row 0 (fully masked, 0/0) is repaired with a 4-column memset.
  - W1/W2 are loaded once per layer (bf16, resident), not once per sequence.
  - Software pipeline attn(b) / ffn(b-1) so the FFN matmul stream covers the
    softmax chain latency of the next sequence; PSUM split 3 stream / 3
    scores / 2 ctx banks to avoid cross-phase false serialization.
"""

import sys

for _p in ("/opt/trn_rl_repo",):
    if _p not in sys.path:
        sys.path.insert(0, _p)

from collections import deque

import numpy as np

import concourse.bacc as bacc
import concourse.bass as bass
import concourse.tile as tile
import concourse.mybir as mybir
from concourse.masks import make_identity

import concourse.tile_utils as tile_utils

tile_utils.max_sbuf_usage = 208 * 1024

F32 = mybir.dt.float32
F32R = mybir.dt.float32r
BF16 = mybir.dt.bfloat16
AF = mybir.ActivationFunctionType
ALU = mybir.AluOpType

P = 128
S, D, H, FF = 512, 512, 8, 2048
DK = D // H  # 64
NT = S // P  # 4
DT = D // P  # 4
NKF = FF // P  # 16
EPS = 1e-5
NEG_BIG = -1e30
N_CORES = 8
PB = [0, 512, 896, 1152]  # packed column base per j-tile (cols i >= tj*128)
PACK_W = 1280


def build_fast(L=4, NB=4):
    nc = bacc.Bacc("TRN2", target_bir_lowering=False, debug=False,
                   num_devices=N_CORES)

    x0_d = nc.dram_tensor("x0", [NB, S, D], F32R, kind="ExternalInput")
    yt_d = nc.dram_tensor("yT", [NB, D, S], BF16, kind="ExternalInput")
    tf_d = nc.dram_tensor("tefr", [NB, P, PACK_W], BF16, kind="ExternalInput")
    wk_d = nc.dram_tensor("Wk", [L, D, D], BF16, kind="ExternalInput")
    wv_d = nc.dram_tensor("Wv", [L, D, D], BF16, kind="ExternalInput")
    wo_d = nc.dram_tensor("Wo", [L, D, D], BF16, kind="ExternalInput")
    w1_d = nc.dram_tensor("W1", [L, D, FF], BF16, kind="ExternalInput")
    w2_d = nc.dram_tensor("W2", [L, FF, D], BF16, kind="ExternalInput")
    out_d = nc.dram_tensor("out", [NB, S, D], F32R, kind="ExternalOutput")

    with tile.TileContext(nc) as tc:
        with (
            tc.tile_pool(name="const", bufs=1) as constp,
            tc.tile_pool(name="state", bufs=18) as statep,
            tc.tile_pool(name="ytp", bufs=NB) as ytp,
            tc.tile_pool(name="tfp", bufs=NB) as tfp,
            tc.tile_pool(name="bfp", bufs=5) as bfp,
            tc.tile_pool(name="medp", bufs=3) as medp,
            tc.tile_pool(name="spp", bufs=2) as spp,
            tc.tile_pool(name="etp", bufs=5) as etp,
            tc.tile_pool(name="htp", bufs=18) as htp,
            tc.tile_pool(name="w3p", bufs=6) as w3p,
            tc.tile_pool(name="w1p", bufs=1) as w1p,
            tc.tile_pool(name="w2p", bufs=1) as w2p,
            tc.tile_pool(name="smallp", bufs=8) as smallp,
            tc.tile_pool(name="denp", bufs=2) as denp,
            tc.tile_pool(name="pstr", bufs=3, space="PSUM") as pstr,
            tc.tile_pool(name="psS", bufs=3, space="PSUM") as psS,
            tc.tile_pool(name="psC", bufs=2, space="PSUM") as psC,
        ):
            ident_f = constp.tile([P, P], F32, tag="identf")
            make_identity(nc, ident_f)
            ident_frt = constp.tile([P, P], F32R, tag="identfr")
            nc.scalar.copy(out=ident_frt[:], in_=ident_f[:])
            ident_fr = ident_frt[:]
            eps_t = constp.tile([P, 1], F32, tag="eps")
            nc.vector.memset(eps_t, EPS)
            ones32 = constp.tile([P, NT * H], F32, tag="ones32")
            nc.vector.memset(ones32, 1.0)

            def transpose_to_bf16(src_of_it, use_vector=False):
                """[i, d] f32r tiles -> one [128, DT, 512] bf16 tile [d, i]."""
                dst = bfp.tile([P, DT, S], BF16, tag="bf", name="tpose")
                for c in range(DT):
                    ps = pstr.tile([P, S], F32R, tag="ps", name="tps")
                    for it in range(NT):
                        nc.tensor.transpose(
                            ps[:, it * P:(it + 1) * P],
                            src_of_it(it)[:, c * P:(c + 1) * P],
                            ident_fr,
                        )
                    if use_vector:
                        nc.vector.tensor_copy(out=dst[:, c, :], in_=ps[:])
                    else:
                        nc.scalar.copy(out=dst[:, c, :], in_=ps[:])
                return dst

            def ln_apply(t, rowsum, dst):
                """LayerNorm over free dim (unit gain / zero bias)."""
                mean_neg = smallp.tile([P, 1], F32, tag="mneg")
                nc.scalar.mul(out=mean_neg, in_=rowsum, mul=-1.0 / D)
                var_s = smallp.tile([P, 1], F32, tag="vars")
                nc.scalar.activation(out=dst, in_=t, func=AF.Square,
                                     bias=mean_neg, scale=1.0,
                                     accum_out=var_s)
                std = smallp.tile([P, 1], F32, tag="std")
                nc.scalar.activation(out=std, in_=var_s, func=AF.Sqrt,
                                     bias=eps_t, scale=1.0 / D)
                rstd = smallp.tile([P, 1], F32, tag="rstd")
                nc.vector.reciprocal(out=rstd, in_=std)
                nc.vector.tensor_scalar(out=dst, in0=t, scalar1=mean_neg,
                                        scalar2=rstd, op0=ALU.add,
                                        op1=ALU.mult)

            # ============ per-sequence init ============
            x_tiles = {}
            yts = {}
            tfs = {}
            for b in range(NB):
                xb = []
                for it in range(NT):
                    xt = statep.tile([P, D], F32R, tag="x", name="xt")
                    nc.sync.dma_start(out=xt[:],
                                      in_=x0_d[b, it * P:(it + 1) * P, :])
                    xb.append(xt)
                x_tiles[b] = xb
                yT = ytp.tile([P, DT, S], BF16, tag="yt", name="yT")
                nc.gpsimd.dma_start(
                    out=yT[:], in_=yt_d[b].rearrange("(c p) i -> p c i", p=P))
                yts[b] = yT
                tf = tfp.tile([P, PACK_W], BF16, tag="tf", name="tf")
                nc.gpsimd.dma_start(out=tf[:], in_=tf_d[b])
                tfs[b] = tf

            def emit_scores(b, h, qkT, tf):
                hp0 = (h % 2) * DK
                qh = qkT[hp0:hp0 + DK, h // 2, :]
                sp = spp.tile([P, PACK_W], BF16, tag="sp", name="sp")
                for tj in range(NT):
                    i0 = tj * P
                    ni = S - i0
                    base = PB[tj]
                    sc = psS.tile([P, S], F32, tag="sc", name="sc")
                    nc.tensor.matmul(sc[:, 0:ni], qh[:, i0:i0 + P],
                                     qh[:, i0:S], start=True, stop=True)
                    nc.vector.tensor_mul(out=sp[:, base:base + ni],
                                         in0=sc[:, 0:ni],
                                         in1=tf[:, base:base + ni])
                    # strict causal mask on the diagonal block: keep j < i,
                    # i.e. partition p < free f
                    nc.gpsimd.affine_select(
                        out=sp[:, base:base + P], in_=sp[:, base:base + P],
                        compare_op=ALU.is_gt, fill=NEG_BIG,
                        base=0, channel_multiplier=-1, pattern=[[1, P]])
                eT = etp.tile([P, PACK_W], BF16, tag="et", name="eT")
                nc.scalar.activation(out=eT[:], in_=sp[:], func=AF.Exp)
                return eT

            def emit_pv(b, h, eT, vext, ctxT):
                hp0 = (h % 2) * DK
                ctx = psC.tile([P, S], F32, tag="ctx", name="ctx")
                for tj in range(NT):
                    i0 = tj * P
                    ni = S - i0
                    base = PB[tj]
                    nc.tensor.matmul(ctx[:, i0:S], vext[:, tj, h, :],
                                     eT[:, base:base + ni],
                                     start=(tj == 0), stop=(tj == NT - 1))
                denr = denp.tile([DK, S], F32, tag="drw", name="denr",
                                 bufs=2)
                nc.scalar.copy(out=denr[:], in_=ctx[DK:P, :])
                rden = denp.tile([DK, S], F32, tag="den", name="rden")
                nc.vector.reciprocal_approx_fast(out=rden[:], in_=denr[:])
                nc.vector.tensor_mul(out=ctxT[hp0:hp0 + DK, h // 2, :],
                                     in0=ctx[0:DK, :], in1=rden[:])

            vext_init = [0]

            def emit_qkv(b, wk, wv):
                xb = x_tiles[b]
                yT = yts[b]
                # v-proj first: it needs only the resident yT, so the PE
                # works while the previous tail's scalar backlog drains;
                # the xT-transpose psum copies then land without ring stalls.
                vext = bfp.tile([P, NT, H, 2 * DK], BF16, tag="vx",
                                name="vext", bufs=2)
                if vext_init[0] < 2:
                    vext_init[0] += 1
                    nc.vector.memset(vext[:, :, :, DK:], 1.0)
                for it in range(NT):
                    ps = pstr.tile([P, S], F32, tag="ps", name="vps")
                    for c in range(DT):
                        nc.tensor.matmul(ps[:],
                                         yT[:, c, it * P:(it + 1) * P],
                                         wv[:, c, :], start=(c == 0),
                                         stop=(c == DT - 1))
                    nc.vector.tensor_copy(
                        out=vext[:, it, :, 0:DK],
                        in_=ps[:].rearrange("p (h k) -> p h k", h=H))
                xT = transpose_to_bf16(lambda it: xb[it])
                qkT = bfp.tile([P, DT, S], BF16, tag="bf", name="qkT")
                for mt in range(DT):
                    ps = pstr.tile([P, S], F32, tag="ps", name="qps")
                    for c in range(DT):
                        nc.tensor.matmul(ps[:],
                                         wk[:, c, mt * P:(mt + 1) * P],
                                         xT[:, c, :], start=(c == 0),
                                         stop=(c == DT - 1))
                    nc.scalar.copy(out=qkT[:, mt, :], in_=ps[:])
                return qkT, vext

            def emit_ffn1_kf(x1T, w1, kf, hts):
                hps = pstr.tile([P, S], F32, tag="ps", name="hps")
                for c in range(DT):
                    nc.tensor.matmul(hps[:],
                                     w1[:, c, kf * P:(kf + 1) * P],
                                     x1T[:, c, :], start=(c == 0),
                                     stop=(c == DT - 1))
                hT = htp.tile([P, S], BF16, tag="ht", name="hT")
                if kf % 2 == 0:
                    nc.scalar.activation(out=hT[:], in_=hps[:], func=AF.Relu)
                else:
                    nc.vector.tensor_scalar_max(out=hT[:], in0=hps[:],
                                                scalar1=0.0)
                hts.append(hT)

            def emit_y2_block(pb, pl, it, hts, w2t, x1b, x2b):
                y2 = pstr.tile([P, S], F32, tag="ps", name="y2ps")
                for kf in range(NKF):
                    nc.tensor.matmul(y2[:],
                                     hts[kf][:, it * P:(it + 1) * P],
                                     w2t[:, kf, :], start=(kf == 0),
                                     stop=(kf == NKF - 1))
                t2 = medp.tile([P, D], F32, tag="med", name="t2")
                rs2 = smallp.tile([P, 1], F32, tag="rs")
                nc.vector.scalar_tensor_tensor(
                    out=t2[:], in0=y2[:], scalar=1.0, in1=x1b[it][:],
                    op0=ALU.mult, op1=ALU.add, accum_out=rs2)
                x2 = statep.tile([P, D], F32R, tag="x", name="x2")
                ln_apply(t2[:], rs2[:], x2[:])
                x2b.append(x2)
                if pl == L - 1:
                    nc.gpsimd.dma_start(
                        out=out_d[pb, it * P:(it + 1) * P, :], in_=x2[:])

            def emit_part2(b, ctxT, wo):
                xb = x_tiles[b]
                # query row 0 is fully masked: den==0 -> 0/0; reference zeroes
                # the attention row, so ctx row i=0 must be 0.
                nc.vector.memset(ctxT[:, :, 0:1], 0.0)
                x1b = []
                for it in range(NT):
                    ps = pstr.tile([P, S], F32, tag="ps", name="ops")
                    for c in range(DT):
                        nc.tensor.matmul(ps[:],
                                         ctxT[:, c, it * P:(it + 1) * P],
                                         wo[:, c, :], start=(c == 0),
                                         stop=(c == DT - 1))
                    t = medp.tile([P, D], F32, tag="med", name="t1")
                    rs = smallp.tile([P, 1], F32, tag="rs")
                    nc.vector.scalar_tensor_tensor(
                        out=t[:], in0=ps[:], scalar=1.0, in1=xb[it][:],
                        op0=ALU.mult, op1=ALU.add, accum_out=rs)
                    x1 = statep.tile([P, D], F32R, tag="x", name="x1")
                    ln_apply(t[:], rs[:], x1[:])
                    x1b.append(x1)
                x_tiles[b] = x1b

            def load_ffn_w(l):
                w1 = w1p.tile([P, DT, FF], BF16, tag="w1", name="w1")
                nc.sync.dma_start(
                    out=w1[:], in_=w1_d[l].rearrange("(c p) f -> p c f", p=P))
                w2t = w2p.tile([P, NKF, D], BF16, tag="w2", name="w2t")
                nc.sync.dma_start(
                    out=w2t[:], in_=w2_d[l].rearrange("(k p) d -> p k d", p=P))
                return w1, w2t

            # ============ layers: fine-grained attn(b) / ffn(prev) weave ====
            # PE stream per sequence: qkv -> (scores(h) + 2 ffn1 blocks)x8
            # with PV(h-3) woven in -> (y2 block + PV drain)x4 -> out-proj.
            # The previous sequence's FFN matmuls pace the softmax chains so
            # the PE never idles and stays at max p-state.
            prev = None  # (b, l, w1, w2t)
            w1c = w2c = None
            for l in range(L):
                wk = w3p.tile([P, DT, D], BF16, tag="w3", name="wk")
                nc.sync.dma_start(
                    out=wk[:], in_=wk_d[l].rearrange("(c p) m -> p c m", p=P))
                wv = w3p.tile([P, DT, D], BF16, tag="w3", name="wv")
                nc.sync.dma_start(
                    out=wv[:], in_=wv_d[l].rearrange("(c p) m -> p c m", p=P))
                wo = w3p.tile([P, DT, D], BF16, tag="w3", name="wo")
                nc.sync.dma_start(
                    out=wo[:], in_=wo_d[l].rearrange("(c p) m -> p c m", p=P))
                if l == 0:
                    w1c, w2c = load_ffn_w(0)
                for b in range(NB):
                    tf = tfs[b]
                    qkT, vext = emit_qkv(b, wk, wv)
                    if prev is not None:
                        pb, pl, pw1, pw2 = prev
                        px1b = x_tiles[pb]
                        px1T = transpose_to_bf16(lambda it: px1b[it])
                        phts = []
                    ctxT = bfp.tile([P, DT, S], BF16, tag="bf", name="ctxT")
                    eTs = []
                    for h in range(H):
                        eTs.append(emit_scores(b, h, qkT, tf))
                        if prev is not None:
                            emit_ffn1_kf(px1T, pw1, 2 * h, phts)
                            emit_ffn1_kf(px1T, pw1, 2 * h + 1, phts)
                        if h >= 3:
                            emit_pv(b, h - 3, eTs[h - 3], vext, ctxT)
                    px2b = []
                    for it in range(NT):
                        if prev is not None:
                            emit_y2_block(pb, pl, it, phts, pw2, px1b, px2b)
                        if it < 3:
                            emit_pv(b, H - 3 + it, eTs[H - 3 + it], vext,
                                    ctxT)
                    if prev is not None:
                        x_tiles[pb] = px2b
                        if pl == l - 1:
                            # prev was the last seq of layer l-1: its FFN
                            # weights are consumed; load this layer's.
                            w1c, w2c = load_ffn_w(l)
                    emit_part2(b, ctxT, wo)
                    prev = (b, l, w1c, w2c)
            # drain the last sequence's FFN
            pb, pl, pw1, pw2 = prev
            px1b = x_tiles[pb]
            px1T = transpose_to_bf16(lambda it: px1b[it])
            phts = []
            for kf in range(NKF):
                emit_ffn1_kf(px1T, pw1, kf, phts)
            px2b = []
            for it in range(NT):
                emit_y2_block(pb, pl, it, phts, pw2, px1b, px2b)
            x_tiles[pb] = px2b

    nc.compile()
    return nc


# ================== general (non-fast) fallback: baseline kernel ==========

def _baseline_build(L=4, NB=4, fast=True):
    """Build the per-core Bass kernel. fast=True assumes zero biases and
    unit LN gains (checked by the host against the actual input values)."""
    nc = bacc.Bacc("TRN2", target_bir_lowering=False, debug=False,
                   num_devices=N_CORES)

    q_d = nc.dram_tensor("q", [NB, S, D], F32, kind="ExternalInput")
    qa_d = nc.dram_tensor("qa", [NB, S, D], F32, kind="ExternalInput")
    pid_d = nc.dram_tensor("pid", [NB, S, S], F32, kind="ExternalInput")
    fr_d = nc.dram_tensor("fr", [NB, S], F32, kind="ExternalInput")
    pos_d = nc.dram_tensor("pos", [S, D], F32, kind="ExternalInput")
    wk_d = nc.dram_tensor("Wk", [L, D, D], F32R, kind="ExternalInput")
    wv_d = nc.dram_tensor("Wv", [L, D, D], F32R, kind="ExternalInput")
    wo_d = nc.dram_tensor("Wo", [L, D, D], F32R, kind="ExternalInput")
    w1_d = nc.dram_tensor("W1", [L, D, FF], FFN_DT, kind="ExternalInput")
    w2_d = nc.dram_tensor("W2", [L, FF, D], FFN_DT, kind="ExternalInput")
    if not fast:
        bk_d = nc.dram_tensor("bk", [L, D], F32, kind="ExternalInput")
        bv_d = nc.dram_tensor("bv", [L, D], F32, kind="ExternalInput")
        bo_d = nc.dram_tensor("bo", [L, D], F32, kind="ExternalInput")
        b1_d = nc.dram_tensor("b1", [L, FF], F32, kind="ExternalInput")
        b2_d = nc.dram_tensor("b2", [L, D], F32, kind="ExternalInput")
        g1_d = nc.dram_tensor("g1", [L, D], F32, kind="ExternalInput")
        gb1_d = nc.dram_tensor("gb1", [L, D], F32, kind="ExternalInput")
        g2_d = nc.dram_tensor("g2", [L, D], F32, kind="ExternalInput")
        gb2_d = nc.dram_tensor("gb2", [L, D], F32, kind="ExternalInput")
    out_d = nc.dram_tensor("out", [NB, S, D], F32, kind="ExternalOutput")

    with tile.TileContext(nc) as tc:
        with (
            tc.tile_pool(name="const", bufs=1) as constp,
            tc.tile_pool(name="state", bufs=18 if fast else 12) as statep,
            tc.tile_pool(name="big", bufs=6 if fast else 4) as bigp,
            tc.tile_pool(name="med", bufs=22 if fast else 12) as medp,
            tc.tile_pool(name="w3", bufs=4) as w3p,
            tc.tile_pool(name="w1", bufs=2) as w1p,
            tc.tile_pool(name="w2", bufs=2) as w2p,
            tc.tile_pool(name="small", bufs=8) as smallp,
            tc.tile_pool(name="frsp", bufs=NB) as frsp,
            tc.tile_pool(name="ps", bufs=8, space="PSUM") as psp,
            tc.tile_pool(name="dram", bufs=1, space="DRAM") as dramp,
        ):
            ident = constp.tile([P, P], F32, tag="ident")
            make_identity(nc, ident)
            eps_t = constp.tile([P, 1], F32, tag="eps")
            nc.vector.memset(eps_t, EPS)
            eps37 = constp.tile([P, 1], F32, tag="eps37")
            nc.vector.memset(eps37, 1e-37)
            ones32 = constp.tile([P, NT * H], F32, tag="ones32")
            nc.vector.memset(ones32, 1.0)

            teT_dram = dramp.tile([NB, S, S], F32, tag="teT_d")
            yT_dram = dramp.tile([NB, S, D], F32R, tag="yT_d")

            def big_tile(dt_, cols=D):
                return bigp.tile([P, NT, cols], dt_, tag="big", name="bigt")

            def med_tile(dt_, cols=D):
                return medp.tile([P, cols], dt_, tag="med", name="medt")

            def transpose_512(src_of_it, out_dt):
                """src_of_it(it) -> AP [128, 512] seq-major tiles.
                Returns one [128, NT, 512] tile = transposed (feature-major)."""
                dst = big_tile(out_dt)
                for c in range(DT):
                    ps = psp.tile([P, S], F32, tag="psC")
                    for it in range(NT):
                        nc.tensor.transpose(
                            ps[:, it * P:(it + 1) * P],
                            src_of_it(it)[:, c * P:(c + 1) * P],
                            ident,
                        )
                    nc.scalar.copy(out=dst[:, c, :], in_=ps[:])
                return dst

            def ln_apply(t, rowsum, dst, g_bc=None, b_bc=None):
                """LayerNorm over free dim: t [128,512] f32 (pre-LN values),
                rowsum [128,1] = sum over free. Writes normalized into dst."""
                mean_neg = smallp.tile([P, 1], F32, tag="mneg")
                nc.scalar.mul(out=mean_neg, in_=rowsum, mul=-1.0 / D)
                var_s = smallp.tile([P, 1], F32, tag="vars")
                # dst used as throwaway scratch for the squares
                nc.scalar.activation(out=dst, in_=t, func=AF.Square,
                                     bias=mean_neg, scale=1.0,
                                     accum_out=var_s)
                std = smallp.tile([P, 1], F32, tag="std")
                nc.scalar.activation(out=std, in_=var_s, func=AF.Sqrt,
                                     bias=eps_t, scale=1.0 / D)
                rstd = smallp.tile([P, 1], F32, tag="rstd")
                nc.vector.reciprocal(out=rstd, in_=std)
                nc.vector.tensor_scalar(out=dst, in0=t, scalar1=mean_neg,
                                        scalar2=rstd, op0=ALU.add,
                                        op1=ALU.mult)
                if g_bc is not None:
                    nc.vector.tensor_mul(out=dst, in0=dst, in1=g_bc)
                if b_bc is not None:
                    nc.vector.tensor_add(out=dst, in0=dst, in1=b_bc)

            def bcast_row(src_row, cols=D):
                """Broadcast [1, cols] sbuf row to [128, cols] via PE."""
                onecol = constp.tile([1, P], F32, tag="onecol")
                nc.vector.memset(onecol, 1.0)
                ps = psp.tile([P, cols], F32, tag="psC")
                nc.tensor.matmul(ps[:], onecol[:], src_row, start=True,
                                 stop=True)
                dst = medp.tile([P, cols], F32, tag="bc", bufs=10, name="bct")
                nc.vector.tensor_copy(out=dst[:], in_=ps[:])
                return dst

            # ============ per-sequence init ============
            x_tiles = {}   # b -> list of NT state APs [128, 512] f32 (seq-major)
            frs = {}
            pos_t = big_tile(F32)
            nc.sync.dma_start(
                out=pos_t[:],
                in_=pos_d[:].rearrange("(it p) d -> p it d", p=P))

            for b in range(NB):
                # x = q + pos
                qt = big_tile(F32)
                nc.sync.dma_start(
                    out=qt[:], in_=q_d[b].rearrange("(it p) d -> p it d", p=P))
                xb = []
                for it in range(NT):
                    xt = statep.tile([P, D], F32, tag="x")
                    nc.vector.tensor_add(out=xt[:], in0=qt[:, it, :],
                                         in1=pos_t[:, it, :])
                    xb.append(xt)
                x_tiles[b] = xb

                # y = qa + pos; yT -> DRAM scratch (f32r)
                yt = big_tile(F32)
                nc.sync.dma_start(
                    out=yt[:], in_=qa_d[b].rearrange("(it p) d -> p it d", p=P))
                for it in range(NT):
                    nc.vector.tensor_add(out=yt[:, it, :], in0=yt[:, it, :],
                                         in1=pos_t[:, it, :])
                yT = transpose_512(lambda it: yt[:, it, :], F32R)
                nc.sync.dma_start(
                    out=yT_dram[b].rearrange("(c p) i -> p c i", p=P),
                    in_=yT[:])

                # te = exp(sigmoid(pid)); teT -> DRAM scratch (f32)
                pt = big_tile(F32, S)
                nc.sync.dma_start(
                    out=pt[:],
                    in_=pid_d[b].rearrange("(it p) j -> p it j", p=P))
                for it in range(NT):
                    nc.scalar.activation(out=pt[:, it, :], in_=pt[:, it, :],
                                         func=AF.Sigmoid)
                    nc.scalar.activation(out=pt[:, it, :], in_=pt[:, it, :],
                                         func=AF.Exp)
                teT = transpose_512(lambda it: pt[:, it, :], F32)
                nc.sync.dma_start(
                    out=teT_dram[b].rearrange("(c p) i -> p c i", p=P),
                    in_=teT[:])

                # forget gate, pre-scaled by 1/sqrt(DK)
                ft = frsp.tile([P, NT], F32, tag="frs")
                nc.sync.dma_start(
                    out=ft[:], in_=fr_d[b].rearrange("(t p) -> p t", p=P))
                nc.scalar.mul(out=ft[:], in_=ft[:], mul=1.0 / np.sqrt(DK))
                frs[b] = ft

            # ============ layers ============
            for l in range(L):
                wk = w3p.tile([P, DT, D], F32R, tag="w3")
                nc.sync.dma_start(
                    out=wk[:], in_=wk_d[l].rearrange("(c p) m -> p c m", p=P))
                wv = w3p.tile([P, DT, D], F32R, tag="w3")
                nc.sync.dma_start(
                    out=wv[:], in_=wv_d[l].rearrange("(c p) m -> p c m", p=P))
                wo = w3p.tile([P, DT, D], F32R, tag="w3")
                nc.sync.dma_start(
                    out=wo[:], in_=wo_d[l].rearrange("(c p) m -> p c m", p=P))

                if not fast:
                    bk_sb = smallp.tile([P, DT], F32, tag="bk")
                    nc.sync.dma_start(
                        out=bk_sb[:],
                        in_=bk_d[l].rearrange("(c p) -> p c", p=P))
                    row = smallp.tile([1, D], F32, tag="brow", bufs=2)
                    nc.sync.dma_start(out=row[:], in_=bv_d[l][None, :])
                    bv_bc = bcast_row(row[:])
                    row2 = smallp.tile([1, D], F32, tag="brow", bufs=2)
                    nc.sync.dma_start(out=row2[:], in_=bo_d[l][None, :])
                    bo_bc = bcast_row(row2[:])
                    row3 = smallp.tile([1, D], F32, tag="brow", bufs=2)
                    nc.sync.dma_start(out=row3[:], in_=b2_d[l][None, :])
                    b2_bc = bcast_row(row3[:])
                    b1_sb = smallp.tile([P, NKF], F32, tag="b1")
                    nc.sync.dma_start(
                        out=b1_sb[:],
                        in_=b1_d[l].rearrange("(c p) -> p c", p=P))
                    rg1 = smallp.tile([1, D], F32, tag="brow", bufs=2)
                    nc.sync.dma_start(out=rg1[:], in_=g1_d[l][None, :])
                    g1_bc = bcast_row(rg1[:])
                    rb1 = smallp.tile([1, D], F32, tag="brow", bufs=2)
                    nc.sync.dma_start(out=rb1[:], in_=gb1_d[l][None, :])
                    gb1_bc = bcast_row(rb1[:])
                    rg2 = smallp.tile([1, D], F32, tag="brow", bufs=2)
                    nc.sync.dma_start(out=rg2[:], in_=g2_d[l][None, :])
                    g2_bc = bcast_row(rg2[:])
                    rb2 = smallp.tile([1, D], F32, tag="brow", bufs=2)
                    nc.sync.dma_start(out=rb2[:], in_=gb2_d[l][None, :])
                    gb2_bc = bcast_row(rb2[:])
                else:
                    bk_sb = None
                    bv_bc = bo_bc = b2_bc = None
                    b1_sb = None
                    g1_bc = gb1_bc = g2_bc = gb2_bc = None

                # -------- attention phase --------
                def emit_scores(b, qkT, teT, h):
                    hp0 = (h % 2) * DK
                    qh = qkT[hp0:hp0 + DK, h // 2, :]
                    eTs = []
                    for tj in range(NT):
                        i0 = tj * P
                        ni = S - i0
                        sc_ps = psp.tile([P, S], F32, tag="psC", name="scps")
                        nc.tensor.matmul(
                            sc_ps[:, 0:ni], qh[:, i0:i0 + P], qh[:, i0:S],
                            start=True, stop=True)
                        sp = med_tile(F32)
                        nc.vector.scalar_tensor_tensor(
                            out=sp[:, 0:ni], in0=sc_ps[:, 0:ni],
                            scalar=frs[b][:, tj:tj + 1],
                            in1=teT[:, tj, i0:S],
                            op0=ALU.mult, op1=ALU.mult)
                        # strict causal mask on the diagonal block:
                        # keep j < i, i.e. partition p < free f
                        nc.gpsimd.affine_select(
                            out=sp[:, 0:P], in_=sp[:, 0:P],
                            compare_op=ALU.is_gt, fill=NEG_BIG,
                            base=0, channel_multiplier=-1,
                            pattern=[[1, P]])
                        eT = med_tile(F32R)
                        nc.scalar.activation(out=eT[:, 0:ni],
                                             in_=sp[:, 0:ni], func=AF.Exp)
                        eTs.append(eT)
                    return eTs

                def emit_pv(vext, ctxT, h, eTs):
                    hp0 = (h % 2) * DK
                    ctx_ps = psp.tile([P, S], F32, tag="psC", name="ctxps")
                    for tj in range(NT):
                        i0 = tj * P
                        ni = S - i0
                        nc.tensor.matmul(
                            ctx_ps[0:DK + 1, i0:S],
                            vext[:, tj, h, :], eTs[tj][:, 0:ni],
                            start=(tj == 0), stop=(tj == NT - 1))
                    dtmp = smallp.tile([1, S], F32, tag="dtmp", bufs=2)
                    nc.scalar.activation(
                        out=dtmp[:], in_=ctx_ps[DK:DK + 1, :],
                        func=AF.Identity, bias=eps37[0:1])
                    denB = smallp.tile([DK, S], F32, tag="dinvB", bufs=3)
                    nc.gpsimd.partition_broadcast(denB[:], dtmp[:])
                    dinvB = smallp.tile([DK, S], F32, tag="dinvB", bufs=3)
                    nc.vector.reciprocal_approx_fast(out=dinvB[:],
                                                     in_=denB[:])
                    nc.vector.tensor_mul(
                        out=ctxT[hp0:hp0 + DK, h // 2, :],
                        in0=ctx_ps[0:DK, :], in1=dinvB[:])

                for b in range(NB):
                    xb = x_tiles[b]
                    # prefetch the per-seq DRAM-scratch streams first so the
                    # DMAs overlap the transpose + projection matmuls
                    teT = big_tile(F32, S)
                    nc.gpsimd.dma_start(
                        out=teT[:],
                        in_=teT_dram[b].rearrange("(c p) i -> p c i", p=P))
                    yT = big_tile(F32R)
                    nc.gpsimd.dma_start(
                        out=yT[:],
                        in_=yT_dram[b].rearrange("(c p) i -> p c i", p=P))
                    xT = transpose_512(lambda it: xb[it], F32R)

                    # qkT[d, i] feature-major
                    qkT = big_tile(F32R)
                    for mt in range(DT):
                        ps = psp.tile([P, S], F32, tag="psC")
                        for c in range(DT):
                            nc.tensor.matmul(
                                ps[:], wk[:, c, mt * P:(mt + 1) * P],
                                xT[:, c, :], start=(c == 0),
                                stop=(c == DT - 1))
                        if bk_sb is not None:
                            nc.scalar.activation(
                                out=qkT[:, mt, :], in_=ps[:],
                                func=AF.Identity, bias=bk_sb[:, mt:mt + 1])
                        else:
                            nc.scalar.copy(out=qkT[:, mt, :], in_=ps[:])

                    # v seq-major with ones column per head: [128, it, h, 65]
                    vext = bigp.tile([P, NT, H, DK + 1], F32R, tag="big")
                    nc.scalar.copy(
                        out=vext[:, :, :, DK:DK + 1],
                        in_=ones32[:].rearrange("p (a b o) -> p a b o",
                                                a=NT, b=H, o=1))
                    for it in range(NT):
                        ps = psp.tile([P, S], F32, tag="psC")
                        for c in range(DT):
                            nc.tensor.matmul(
                                ps[:], yT[:, c, it * P:(it + 1) * P],
                                wv[:, c, :], start=(c == 0),
                                stop=(c == DT - 1))
                        pv = ps[:].rearrange("p (h k) -> p h k", h=H)
                        if bv_bc is not None:
                            nc.vector.scalar_tensor_tensor(
                                out=vext[:, it, :, 0:DK], in0=pv, scalar=1.0,
                                in1=bv_bc[:].rearrange("p (h k) -> p h k",
                                                       h=H),
                                op0=ALU.mult, op1=ALU.add)
                        else:
                            nc.vector.tensor_copy(out=vext[:, it, :, 0:DK],
                                                  in_=pv)

                    ctxT = big_tile(F32R)
                    from collections import deque
                    pending = deque()
                    for h in range(H):
                        pending.append((h, emit_scores(b, qkT, teT, h)))
                        if len(pending) > 3:
                            ph, peTs = pending.popleft()
                            emit_pv(vext, ctxT, ph, peTs)
                    while pending:
                        ph, peTs = pending.popleft()
                        emit_pv(vext, ctxT, ph, peTs)

                    # out-proj + residual + LN1
                    x1b = []
                    for it in range(NT):
                        ps = psp.tile([P, S], F32, tag="psC")
                        for c in range(DT):
                            nc.tensor.matmul(
                                ps[:], ctxT[:, c, it * P:(it + 1) * P],
                                wo[:, c, :], start=(c == 0),
                                stop=(c == DT - 1))
                        t = med_tile(F32)
                        rs = smallp.tile([P, 1], F32, tag="rs")
                        if bo_bc is not None:
                            nc.vector.scalar_tensor_tensor(
                                out=t[:], in0=ps[:], scalar=1.0, in1=bo_bc[:],
                                op0=ALU.mult, op1=ALU.add)
                            nc.vector.scalar_tensor_tensor(
                                out=t[:], in0=t[:], scalar=1.0, in1=xb[it][:],
                                op0=ALU.mult, op1=ALU.add, accum_out=rs)
                        else:
                            nc.vector.scalar_tensor_tensor(
                                out=t[:], in0=ps[:], scalar=1.0,
                                in1=xb[it][:],
                                op0=ALU.mult, op1=ALU.add, accum_out=rs)
                        x1 = statep.tile([P, D], F32, tag="x")
                        ln_apply(t[:], rs[:], x1[:], g1_bc and g1_bc[:],
                                 gb1_bc and gb1_bc[:])
                        x1b.append(x1)
                    x_tiles[b] = x1b

                # -------- FFN phase --------
                for b in range(NB):
                    x1b = x_tiles[b]
                    x1T = transpose_512(lambda it: x1b[it], FFN_DT)
                    y2_ps = [psp.tile([P, S], F32, tag="psC", name="y2ps")
                             for _i in range(NT)]
                    pend_ffn2 = []
                    for g in range(NKF // 4):
                        w1g = w1p.tile([P, DT, 4 * P], FFN_DT, tag="w1")
                        nc.sync.dma_start(
                            out=w1g[:],
                            in_=w1_d[l].rearrange("(c p) f -> p c f",
                                                  p=P)[:, :,
                                                       g * 512:(g + 1) * 512])
                        w2g = w2p.tile([P, 4, D], FFN_DT, tag="w2")
                        nc.sync.dma_start(
                            out=w2g[:],
                            in_=w2_d[l].rearrange("(c p) d -> p c d",
                                                  p=P)[:, 4 * g:4 * g + 4, :])
                        for j in range(4):
                            kf = 4 * g + j
                            h_ps = psp.tile([P, S], F32, tag="psC")
                            for c in range(DT):
                                nc.tensor.matmul(
                                    h_ps[:], w1g[:, c, j * P:(j + 1) * P],
                                    x1T[:, c, :], start=(c == 0),
                                    stop=(c == DT - 1))
                            hT = med_tile(FFN_DT)
                            if b1_sb is not None:
                                nc.scalar.activation(
                                    out=hT[:], in_=h_ps[:], func=AF.Relu,
                                    bias=b1_sb[:, kf:kf + 1])
                            elif kf % 2 == 0:
                                nc.scalar.activation(out=hT[:], in_=h_ps[:],
                                                     func=AF.Relu)
                            else:
                                nc.vector.tensor_scalar_max(
                                    out=hT[:], in0=h_ps[:], scalar1=0.0)
                            # pipeline by two kf: ffn2(kf-2) is emitted
                            # after ffn1(kf) so the PE isn't stalled on relu
                            pend_ffn2.append((hT, w2g, j, kf))
                            if len(pend_ffn2) > 2:
                                phT, pw2g, pj, pkf = pend_ffn2.pop(0)
                                for it in range(NT):
                                    nc.tensor.matmul(
                                        y2_ps[it][:],
                                        phT[:, it * P:(it + 1) * P],
                                        pw2g[:, pj, :], start=(pkf == 0),
                                        stop=(pkf == NKF - 1))
                    for phT, pw2g, pj, pkf in pend_ffn2:
                        for it in range(NT):
                            nc.tensor.matmul(
                                y2_ps[it][:], phT[:, it * P:(it + 1) * P],
                                pw2g[:, pj, :], start=(pkf == 0),
                                stop=(pkf == NKF - 1))
                    x2b = []
                    for it in range(NT):
                        t2 = med_tile(F32)
                        rs2 = smallp.tile([P, 1], F32, tag="rs")
                        if b2_bc is not None:
                            nc.vector.scalar_tensor_tensor(
                                out=t2[:], in0=y2_ps[it][:], scalar=1.0,
                                in1=b2_bc[:], op0=ALU.mult, op1=ALU.add)
                            nc.vector.scalar_tensor_tensor(
                                out=t2[:], in0=t2[:], scalar=1.0,
                                in1=x1b[it][:], op0=ALU.mult, op1=ALU.add,
                                accum_out=rs2)
                        else:
                            nc.vector.scalar_tensor_tensor(
                                out=t2[:], in0=y2_ps[it][:], scalar=1.0,
                                in1=x1b[it][:], op0=ALU.mult, op1=ALU.add,
                                accum_out=rs2)
                        x2 = statep.tile([P, D], F32, tag="x")
                        ln_apply(t2[:], rs2[:], x2[:], g2_bc and g2_bc[:],
                                 gb2_bc and gb2_bc[:])
                        x2b.append(x2)
                    x_tiles[b] = x2b
                    if l == L - 1:
                        for it in range(NT):
                            nc.sync.dma_start(
                                out=out_d[b, it * P:(it + 1) * P, :],
                                in_=x2b[it][:])

    nc.compile()
    return nc


def build_general(L=4, NB=4):
    return _baseline_build(L, NB, fast=False)


_BUILD_CACHE = {}


def _get_nc(L, NB, fast):
    key = (L, NB, fast)
    if key not in _BUILD_CACHE:
        _BUILD_CACHE[key] = (build_fast(L, NB) if fast
                             else build_general(L, NB))
    return _BUILD_CACHE[key]


def _is_fast(w):
    return (all(np.all(np.asarray(w[n]) == 0.0) for n in
                ["bk", "bv", "bo", "b1", "b2", "ln1_b", "ln2_b"])
            and all(np.all(np.asarray(w[n]) == 1.0)
                    for n in ["ln1_g", "ln2_g"]))


def make_in_maps(inputs, L=4, NB=4, n_cores=N_CORES):
    """Shard full inputs into per-core in_maps. Returns (in_maps, fast)."""
    import ml_dtypes
    bf = ml_dtypes.bfloat16
    f32 = np.float32
    fast = _is_fast(inputs)
    if not fast:
        return _baseline_make_in_maps(inputs, L=L, NB=NB, n_cores=n_cores)

    q = np.asarray(inputs["q_embed_data"], f32)
    qa = np.asarray(inputs["qa_embed_data"], f32)
    pid = np.asarray(inputs["pid_embed_data"], f32)
    fr = np.asarray(inputs["forget_rate"], f32)[:, :, 0]
    pos = np.asarray(inputs["pos_emb"], f32).reshape(S, D)
    B = q.shape[0]

    x0 = np.ascontiguousarray(q + pos[None])
    yT = np.ascontiguousarray(
        np.swapaxes(qa + pos[None], 1, 2)).astype(bf)
    te = np.clip(np.exp(1.0 / (1.0 + np.exp(-pid))), 1e-5, 1e5)
    # tefr[b, j, i] = te[b, i, j] * fr[b, j] / sqrt(DK), causally packed
    tf_ji = np.swapaxes(te, 1, 2) * (fr[:, :, None] * (1.0 / np.sqrt(DK)))
    tfP = np.zeros((B, P, PACK_W), f32)
    for tj in range(NT):
        i0 = tj * P
        tfP[:, :, PB[tj]:PB[tj] + S - i0] = \
            tf_ji[:, i0:i0 + P, i0:].astype(f32)
    tfP = np.ascontiguousarray(tfP).astype(bf)

    wts = {n: np.ascontiguousarray(np.asarray(inputs[n], f32)[:L]).astype(bf)
           for n in ["Wk", "Wv", "Wo", "W1", "W2"]}

    in_maps = []
    for c in range(n_cores):
        sl = slice(c * NB, (c + 1) * NB)
        m = {
            "x0": x0[sl], "yT": yT[sl], "tefr": tfP[sl],
            "Wk": wts["Wk"], "Wv": wts["Wv"], "Wo": wts["Wo"],
            "W1": wts["W1"], "W2": wts["W2"],
        }
        in_maps.append(m)
    return in_maps, fast


def kernel(**inputs):
    from concourse.bass_utils import run_bass_kernel_spmd

    B = int(np.asarray(inputs["q_embed_data"]).shape[0])
    NB = B // N_CORES
    L = int(np.asarray(inputs["Wk"]).shape[0])
    in_maps, fast = make_in_maps(inputs, L=L, NB=NB)
    nc = _get_nc(L, NB, fast)
    res = run_bass_kernel_spmd(nc, in_maps, core_ids=list(range(N_CORES)))
    out = np.concatenate([res.results[c]["out"] for c in range(N_CORES)],
                         axis=0)
    return out.astype(np.float32)


def _baseline_make_in_maps(inputs, L=4, NB=4, n_cores=N_CORES):
    """Shard full inputs into per-core in_maps. Returns (in_maps, fast)."""
    f32 = np.float32
    q = np.ascontiguousarray(np.asarray(inputs["q_embed_data"], f32))
    qa = np.ascontiguousarray(np.asarray(inputs["qa_embed_data"], f32))
    pid = np.ascontiguousarray(np.asarray(inputs["pid_embed_data"], f32))
    fr = np.ascontiguousarray(np.asarray(inputs["forget_rate"], f32)[:, :, 0])
    pos = np.ascontiguousarray(np.asarray(inputs["pos_emb"], f32)[0])
    names = ["Wk", "bk", "Wv", "bv", "Wo", "bo", "ln1_g", "ln1_b", "W1", "b1",
             "W2", "b2", "ln2_g", "ln2_b"]
    w = {n: np.ascontiguousarray(np.asarray(inputs[n], f32)) for n in names}
    if FFN_BF16:
        import ml_dtypes
        w["W1"] = w["W1"].astype(ml_dtypes.bfloat16)
        w["W2"] = w["W2"].astype(ml_dtypes.bfloat16)

    fast = (all(np.all(w[n] == 0.0) for n in
                ["bk", "bv", "bo", "b1", "b2", "ln1_b", "ln2_b"])
            and all(np.all(w[n] == 1.0) for n in ["ln1_g", "ln2_g"]))

    in_maps = []
    for c in range(n_cores):
        sl = slice(c * NB, (c + 1) * NB)
        m = {
            "q": q[sl], "qa": qa[sl], "pid": pid[sl], "fr": fr[sl],
            "pos": pos,
            "Wk": w["Wk"][:L], "Wv": w["Wv"][:L], "Wo": w["Wo"][:L],
            "W1": w["W1"][:L], "W2": w["W2"][:L],
        }
        if not fast:
            m.update({
                "bk": w["bk"][:L], "bv": w["bv"][:L], "bo": w["bo"][:L],
                "b1": w["b1"][:L], "b2": w["b2"][:L],
                "g1": w["ln1_g"][:L], "gb1": w["ln1_b"][:L],
                "g2": w["ln2_g"][:L], "gb2": w["ln2_b"][:L],
            })
        in_maps.append(m)
    return in_maps, fast
